# revision 43
# baseline (speedup 1.0000x reference)
"""Trainium2 Bass kernel for an nn.Block dense transformer layer.

Reference computation (per batch element b of 8):
    x = x + MHA(LN1(x));  x = x + MLP(LN2(x))
with T=1024 tokens, C=512 channels, H=16 heads (d=32), MLP hidden 2048,
new-gelu (tanh approx), softmax without causal mask.

Sharding: pure data parallelism - each of the 8 NeuronCores processes one
batch element. No collectives.

v2: fp8 (e4m3) DoubleRow matmuls for QKV/Proj/FC/CProj (K=256 per
instruction = 2x PE throughput), fp8 attention weights + values (plain
rate), per-head A.V + denominator as M=32 DR matmuls at partition base 0
with an aligned reciprocal-normalize and a partition-moving DMA into the
proj input layout. Activation-table schedule: natural_log_exp set for
LN1/attention/LN2, one switch to gelu_apprx_tanh. x is DMA'd before the
weights so LN1 starts immediately.

Scale scheme (validated vs reference in fp64/numpy, rel ~1.2e-2 < 2e-2):
  LN outs x16 (folded into ln w/b); weights x4096 (cproj x8192);
  exp out = 8*exp(s) (ln 8 folded into ACT bias); v8 = 16*v;
  dn ones = 0.5 so av/dn = 32*y; descales folded into PSUM-evac ops.
"""

import sys

if "/opt/trn_rl_repo" not in sys.path:
    sys.path.insert(0, "/opt/trn_rl_repo")

import math
from contextlib import ExitStack

import ml_dtypes
import numpy as np

import concourse.bass as bass
import concourse.mybir as mybir
import concourse.tile as tile
from concourse import bacc
from concourse import bass_utils

F32 = mybir.dt.float32
F32R = mybir.dt.float32r
BF16 = mybir.dt.bfloat16
F8 = mybir.dt.float8e4
AF = mybir.ActivationFunctionType
OP = mybir.AluOpType
DR = mybir.MatmulPerfMode.DoubleRow

N_CORES = 8
T = 1024  # tokens
C = 512  # channels
H = 16  # heads
D = 32  # head dim
FF = 2048  # mlp hidden
CT = C // 128  # channel partition tiles (4)
FT = FF // 128  # mlp hidden partition tiles (16)
NQ = T // 512  # token (query) 512-chunks (2)
G = H // 4  # head groups of 4 (4)
EPS = 1e-5
SCALE = 1.0 / math.sqrt(D)

SX = 16.0  # LN-output fp8 scale (folded into ln w/b host-side)
SW = 4096.0  # qkv/proj/fc weight scale
SWC = 8192.0  # cproj weight scale
SA = 8.0  # exp-output scale (ln SA folded into ACT bias)
SV = 16.0  # v fp8 scale
ONES_VAL = 0.5  # dn ones value -> av/dn = (SA*SV)/(SA*ONES_VAL) * y = 32*y
SY = SV / ONES_VAL  # 32
D_QKV = 1.0 / (SX * SW)  # 2^-16
D_V = SV / (SX * SW)  # 2^-12
D_PROJ = 1.0 / (SY * SW)  # 2^-17
D_FC = 1.0 / (SX * SW)  # 2^-16
D_CPROJ = 1.0 / (1.0 * SWC)  # 2^-13 (gelu out stored unscaled)
GELU_FUNC = AF.Gelu_apprx_tanh
# Schraudolph fast-exp on DVE for these key-tiles (offloads softmax exp
# work from ScalarE): bits = ACOEF*s + BCOEF, reinterpreted as fp32 =
# ~8*exp(s*SCALE) within +-4%. Disabled: the fp8 store needs a second
# full-rate pass on DVE/GpSimd, which costs more than ACT saves.
OFF_KTS = ()
SCH_A = float(2.0**23) * math.log2(math.e) * SCALE
SCH_B = float(2.0**23) * (127.0 + math.log2(SA) - 0.0579)


def r32(ap):
    return ap.bitcast(F32R)


class _NS:
    pass


def emit_prep(ctx, nc, tc, io, tag=""):
    """Persistent tiles + weight/const DMAs. x is loaded FIRST."""
    P = _NS()
    wpool = ctx.enter_context(tc.tile_pool(name="w" + tag, bufs=1))

    def single(shape, dtype, t):
        return wpool.tile(shape, dtype, tag=t, name=t)

    # ---- activations (persistent) ----
    x_t = [single([128, T], F32, f"xT{k}") for k in range(CT)]
    a8 = single([128, CT, T], F8, "a8")  # LN out *16, DR-paired layout
    q_t = [single([128, T], BF16, f"qT{g}") for g in range(G)]
    k_t = [single([128, T], BF16, f"kT{g}") for g in range(G)]
    # v8: [token, kt, head, 64] with cols 0:32 = 0.5 (dn-ones), cols
    # 32:64 = v*16, so one M=64 DR matmul yields dn rows 0:32 (reciprocal
    # reads PSUM directly, partition-aligned) + av rows 32:64.
    v8 = single([128, 8, H, 64], F8, "v8")
    av8 = single([128, G, T], F8, "av8")  # y*32, DR-paired for proj

    # x first so LN1 can start while weights stream in
    for k in range(CT):
        nc.sync.dma_start(
            out=x_t[k].bitcast(F32R),
            in_=io["xT"].bitcast(F32R)[128 * k : 128 * (k + 1), :],
        )

    # ---- fp8 weights (DR-paired layout [p, kt, out_features]) ----
    # qkv on the sync queue (needed first); the rest on the scalar-engine
    # queue so both DMA streams run in parallel with LN1 compute.
    w_qkv = single([128, CT, 3 * C], F8, "wqkv8")
    w_proj = single([128, CT, C], F8, "wproj8")
    w_fc = single([128, CT, FF], F8, "wfc8")
    w_cproj = single([128, FT, C], F8, "wcproj8")
    nc.sync.dma_start(out=w_qkv, in_=io["wqkv8"])
    nc.scalar.dma_start(out=w_fc, in_=io["wfc8"])
    nc.scalar.dma_start(out=w_cproj, in_=io["wcproj8"])
    nc.scalar.dma_start(out=w_proj, in_=io["wproj8"])

    # ---- bias / ln columns: tile[p, m] = vec[m*128 + p] ----
    def colmat(dram_ap, ntiles, t):
        tl = single([128, ntiles], F32, t)
        nc.sync.dma_start(out=tl, in_=dram_ap.transpose([1, 0]))
        return tl

    b_qk = colmat(io["bqk"], 8, "bqk")
    b_proj = colmat(io["bproj"], CT, "bproj")
    b_fc = colmat(io["bfc"], FT, "bfc")
    b_cproj = colmat(io["bcproj"], CT, "bcproj")
    ln1w = colmat(io["ln1w"], CT, "ln1w")  # pre-scaled *16 host-side
    ln1b = colmat(io["ln1b"], CT, "ln1b")
    ln2w = colmat(io["ln2w"], CT, "ln2w")
    ln2b = colmat(io["ln2b"], CT, "ln2b")

    # v bias broadcast (*16) to all partitions [128, C]
    bv_bc = single([128, C], F32, "bv_bc")
    nc.gpsimd.dma_start(
        out=bv_bc,
        in_=bass.AP(tensor=io["bv16"].tensor, offset=0, ap=[[0, 128], [1, C]]),
    )

    ones_f = single([128, 128], F32, "ones_f")
    nc.sync.dma_start(out=ones_f.bitcast(F32R), in_=io["ones_d"].bitcast(F32R))
    nc.vector.memset(v8, ONES_VAL)  # evac overwrites the v halves
    eps_t = single([128, 1], F32, "eps_t")
    nc.vector.memset(eps_t, EPS)
    ln_sa = single([128, 1], F32, "ln_sa")
    nc.vector.memset(ln_sa, math.log(SA))

    # rotating pools (SBUF)
    tmp = ctx.enter_context(tc.tile_pool(name="tmp" + tag, bufs=3))
    stat = ctx.enter_context(tc.tile_pool(name="stat" + tag, bufs=2))
    a2p = ctx.enter_context(tc.tile_pool(name="a2p" + tag, bufs=2))
    g8p = ctx.enter_context(tc.tile_pool(name="g8p" + tag, bufs=2))
    rcpp = ctx.enter_context(tc.tile_pool(name="rcp" + tag, bufs=3))
    y8p = ctx.enter_context(tc.tile_pool(name="y8p" + tag, bufs=3))
    dnp = ctx.enter_context(tc.tile_pool(name="dnp" + tag, bufs=2))
    schp = ctx.enter_context(tc.tile_pool(name="schp" + tag, bufs=2))

    for name in ("x_t", "a8", "q_t", "k_t", "v8", "av8", "w_qkv", "w_proj",
                 "w_fc", "w_cproj", "b_qk", "b_proj", "b_fc", "b_cproj",
                 "ln1w", "ln1b", "ln2w", "ln2b", "bv_bc", "ones_f",
                 "eps_t", "ln_sa", "tmp", "stat", "a2p", "g8p", "rcpp", "y8p",
                 "dnp", "schp"):
        setattr(P, name, locals()[name])
    return P


def emit_body(nc, tc, io, P, tag="", reload_x=False):
    p = P
    if reload_x:
        for k in range(CT):
            nc.sync.dma_start(
                out=p.x_t[k].bitcast(F32R),
                in_=io["xT"].bitcast(F32R)[128 * k : 128 * (k + 1), :],
            )

    # ---------------- LayerNorm (transposed domain) -> a8 fp8 -------------
    def layernorm(wcol, bcol, cols, psp, heavy=None):
        """LN over channel (partition) axis of x_t restricted to token
        range `cols`; writes (normalized*16) as fp8 into a8[:, k, cols].
        `heavy` picks the engine for the elementwise square/sub/mult ops
        (vector when latency-critical, gpsimd when DVE is the scarce one).
        """
        heavy = heavy or nc.vector
        ncols = cols.stop - cols.start
        musum = psp.tile([128, ncols], F32, tag="mm", name="ln_mu")
        sqsum = psp.tile([128, ncols], F32, tag="mm", name="ln_sq")
        for k in range(CT):
            sq = p.tmp.tile([128, ncols], F32, tag="sq", name="sq")
            heavy.tensor_tensor(
                out=sq.bitcast(F32R), in0=p.x_t[k][:, cols],
                in1=p.x_t[k][:, cols], op=OP.mult,
            )
            nc.tensor.matmul(
                out=musum, lhsT=r32(p.ones_f), rhs=r32(p.x_t[k][:, cols]),
                start=(k == 0), stop=(k == CT - 1),
            )
            nc.tensor.matmul(
                out=sqsum, lhsT=r32(p.ones_f), rhs=r32(sq),
                start=(k == 0), stop=(k == CT - 1),
            )
        mu = p.stat.tile([128, ncols], F32, tag="mu", name="mu")
        rstd = p.stat.tile([128, ncols], F32, tag="rstd", name="rstd")
        var = p.stat.tile([128, ncols], F32, tag="var", name="var")
        nc.vector.tensor_scalar_mul(out=mu, in0=musum, scalar1=1.0 / C)
        nc.vector.tensor_scalar_mul(out=var, in0=sqsum, scalar1=1.0 / C)
        nc.vector.tensor_tensor(out=rstd, in0=mu, in1=mu, op=OP.mult)
        nc.vector.tensor_tensor(out=var, in0=var, in1=rstd, op=OP.subtract)
        # rstd = exp(-0.5*ln(var+eps)) (stays on natural_log_exp table set)
        nc.scalar.activation(out=var, in_=var, func=AF.Ln, bias=p.eps_t, scale=1.0)
        nc.scalar.activation(out=rstd, in_=var, func=AF.Exp, bias=0.0, scale=-0.5)
        for k in range(CT):
            t1 = p.tmp.tile([128, ncols], F32, tag="t1", name="ln_t1")
            heavy.tensor_tensor(
                out=t1.bitcast(F32R), in0=p.x_t[k][:, cols], in1=mu, op=OP.subtract
            )
            heavy.tensor_tensor(out=t1.bitcast(F32R), in0=t1, in1=rstd, op=OP.mult)
            if k % 2:  # ScalarE is idle during the LN/QKV phases
                nc.scalar.activation(
                    out=p.a8[:, k, cols], in_=t1, func=AF.Identity,
                    bias=bcol[:, k : k + 1], scale=wcol[:, k : k + 1],
                )
            else:
                nc.vector.tensor_scalar(
                    out=p.a8[:, k, cols], in0=t1,
                    scalar1=wcol[:, k : k + 1], scalar2=bcol[:, k : k + 1],
                    op0=OP.mult, op1=OP.add,
                )

    # ============== LN1 + QKV + attention (one PSUM scope) ================
    # Shared bank budget (2 + 2x2 + 2 = 8) lets the scheduler hoist the
    # first attention groups' scores/exp into the QKV phase as soon as
    # their q/k tiles are evacuated (m order 0,4,1,5,... completes head
    # group g after pair (g, 4+g)).
    with tc.tile_pool(name="ps1" + tag, bufs=2, space="PSUM") as pmm, \
         tc.tile_pool(name="sc" + tag, bufs=1, space="PSUM") as scp, \
         tc.tile_pool(name="avdn" + tag, bufs=2, space="PSUM") as avp:
        # q^T, k^T: transposed out (feature on partitions), bf16 + bias.
        # nt-outer so chunk 1's LN overlaps chunk 0's QKV; evacuations
        # alternate DVE / ScalarE (idle here) by m parity.
        for nt in range(NQ):
            layernorm(p.ln1w, p.ln1b, slice(512 * nt, 512 * (nt + 1)), pmm,
                      heavy=nc.vector)
            for m in (0, 4, 1, 5, 2, 6, 3, 7):  # q/k pairs per head group
                dst = p.q_t[m] if m < 4 else p.k_t[m - 4]
                ps = pmm.tile([128, 512], F32, tag="mm", name="qk_ps")
                for j in range(2):
                    nc.tensor.matmul(
                        out=ps,
                        lhsT=p.w_qkv[:, 2 * j : 2 * j + 2, 128 * m : 128 * (m + 1)],
                        rhs=p.a8[:, 2 * j : 2 * j + 2, 512 * nt : 512 * (nt + 1)],
                        start=(j == 0), stop=(j == 1), perf_mode=DR,
                    )
                if m % 2:
                    nc.scalar.activation(
                        out=dst[:, 512 * nt : 512 * (nt + 1)], in_=ps,
                        func=AF.Identity, bias=p.b_qk[:, m : m + 1], scale=D_QKV,
                    )
                else:
                    nc.vector.tensor_scalar(
                        out=dst[:, 512 * nt : 512 * (nt + 1)], in0=ps,
                        scalar1=D_QKV, scalar2=p.b_qk[:, m : m + 1],
                        op0=OP.mult, op1=OP.add,
                    )
            # v natural layout [token, vfeat]: lhsT = a8 token-tile
            for t in range(4 * nt, 4 * nt + 4):
                ps = pmm.tile([128, C], F32, tag="mm", name="v_ps")
                for j in range(2):
                    nc.tensor.matmul(
                        out=ps,
                        lhsT=p.a8[:, 2 * j : 2 * j + 2, 128 * t : 128 * (t + 1)],
                        rhs=p.w_qkv[:, 2 * j : 2 * j + 2, 2 * C : 3 * C],
                        start=(j == 0), stop=(j == 1), perf_mode=DR,
                    )
                nc.vector.scalar_tensor_tensor(
                    out=p.v8[:, t, :, 32:64], in0=ps, scalar=D_V, in1=p.bv_bc,
                    op0=OP.mult, op1=OP.add,
                )

        # ======================== Attention ===============================
        # per (qc, g): scores (bf16, 4-head row-packed) -> exp (fp8, *8)
        # -> A2; per head one M=64 DR matmul gives dn rows 0:31 + av rows
        # 32:63; aligned normalize, DMA into av8 row block.
        sc_ctr = [0]
        # Software pipeline: group g's A.V matmuls are interleaved two-per-
        # kt-iteration into group g+1's scores stream, so ScalarE's exp
        # pipeline never stalls behind a burst of AV work on the PE.
        av_state = {}

        def emit_av_step(a2_prev, g_prev, qs_prev, step):
            h, j = divmod(step, 4)
            hg = 4 * g_prev + h
            if j == 0:
                av_state[h] = avp.tile([64, 512], F32, tag="av", name="av_ps")
            av_ps = av_state[h]
            nc.tensor.matmul(
                out=av_ps,
                lhsT=p.v8[:, 2 * j : 2 * j + 2, hg, :],
                rhs=a2_prev[h // 2][
                    :, 2 * j : 2 * j + 2, 512 * (h % 2) : 512 * (h % 2) + 512,
                ],
                start=(j == 0), stop=(j == 3), perf_mode=DR,
            )
            if j == 3:
                # dn at rows 0:31 -> reciprocal straight off PSUM (aligned);
                # shift rcp to rows 32:63 by DMA; aligned normalize-multiply
                # at rows 32:63; placement DMA into av8's row block.
                rcp = p.rcpp.tile([32, 512], F32, tag="rcp", name="rcp")
                nc.vector.reciprocal_approx_fast(out=rcp, in_=av_ps[0:32, :])
                rcps = p.dnp.tile([64, 512], F32, tag="rcps", name="rcps")
                nc.gpsimd.dma_start(out=rcps[32:64, :], in_=rcp)
                y8s = p.y8p.tile([64, 512], F8, tag="y8", name="y8s")
                nc.vector.tensor_tensor(
                    out=y8s[32:64, :], in0=av_ps[32:64, :],
                    in1=rcps[32:64, :], op=OP.mult,
                )
                nc.gpsimd.dma_start(
                    out=p.av8[32 * h : 32 * h + 32, g_prev, qs_prev],
                    in_=y8s[32:64, :],
                )

        prev = None
        for qc in range(NQ):
            qs = slice(512 * qc, 512 * (qc + 1))
            for g in range(G):
                a2 = [p.a2p.tile([128, 8, 1024], F8, tag=f"a2_{i}", name="a2")
                      for i in range(2)]
                for half in range(2):
                    for kt in range(4):
                        ktg = 4 * half + kt
                        sc = []
                        for i in range(2):
                            t2 = sc_ctr[0] % 2
                            sc_ctr[0] += 1
                            sc.append(scp.tile([128, 1024], F32,
                                               tag=f"sc{t2}", name="sc"))
                        for c in range(4):
                            pr = slice(32 * c, 32 * (c + 1))
                            nc.tensor.matmul(
                                out=sc[c // 2][:, 512 * (c % 2) : 512 * (c % 2 + 1)],
                                lhsT=p.k_t[g][pr, 128 * ktg : 128 * (ktg + 1)],
                                rhs=p.q_t[g][pr, qs],
                                start=True, stop=True,
                                tile_position=(32 * c, 0),
                            )
                        for i in range(2):
                            if ktg in OFF_KTS:
                                ti = p.schp.tile(
                                    [128, 1024], mybir.dt.int32,
                                    tag="sch", name="sch",
                                )
                                nc.vector.tensor_scalar(
                                    out=ti, in0=sc[i], scalar1=SCH_A,
                                    scalar2=SCH_B, op0=OP.mult, op1=OP.add,
                                )
                                nc.gpsimd.tensor_copy(
                                    a2[i][:, ktg, :], ti.bitcast(F32)
                                )
                            else:
                                nc.scalar.activation(
                                    out=a2[i][:, ktg, :], in_=sc[i], func=AF.Exp,
                                    bias=p.ln_sa, scale=SCALE,
                                )
                if prev is not None:
                    for _ in range(16):
                        emit_av_step(*prev)
                        prev = (prev[0], prev[1], prev[2], prev[3] + 1)
                prev = (a2, g, qs, 0)
        # drain the last group's AV work
        for _ in range(16):
            emit_av_step(*prev)
            prev = (prev[0], prev[1], prev[2], prev[3] + 1)

    # =================== proj + residual, LN2, MLP (DR fp8) ===============
    with tc.tile_pool(name="ps2" + tag, bufs=4, space="PSUM") as pmm:
        # proj + LN2 for both chunks first (keeps natural_log_exp loaded),
        # then all gelu work (single switch to the gelu table set).
        for qc in range(NQ):
            qs = slice(512 * qc, 512 * (qc + 1))
            for m in range(CT):
                ps = pmm.tile([128, 512], F32, tag="mm", name="proj_ps")
                for j in range(2):
                    nc.tensor.matmul(
                        out=ps,
                        lhsT=p.w_proj[:, 2 * j : 2 * j + 2, 128 * m : 128 * (m + 1)],
                        rhs=p.av8[:, 2 * j : 2 * j + 2, qs],
                        start=(j == 0), stop=(j == 1), perf_mode=DR,
                    )
                nc.vector.affine_then_add(
                    out=p.x_t[m][:, qs].bitcast(F32R), in0=ps,
                    in1=p.x_t[m][:, qs], scale=D_PROJ,
                    bias=p.b_proj[:, m : m + 1],
                )
            layernorm(p.ln2w, p.ln2b, qs, pmm, heavy=nc.gpsimd)
        for qc in range(NQ):
            qs = slice(512 * qc, 512 * (qc + 1))
            g8 = p.g8p.tile([128, FT, 512], F8, tag="g8", name="g8")
            for m in range(FT):
                ps = pmm.tile([128, 512], F32, tag="mm", name="fc_ps")
                for j in range(2):
                    nc.tensor.matmul(
                        out=ps,
                        lhsT=p.w_fc[:, 2 * j : 2 * j + 2, 128 * m : 128 * (m + 1)],
                        rhs=p.a8[:, 2 * j : 2 * j + 2, qs],
                        start=(j == 0), stop=(j == 1), perf_mode=DR,
                    )
                nc.scalar.activation(
                    out=g8[:, m, :], in_=ps, func=GELU_FUNC,
                    bias=p.b_fc[:, m : m + 1], scale=D_FC,
                )
            for m in range(CT):
                ps = pmm.tile([128, 512], F32, tag="mm", name="cproj_ps")
                for j in range(FT // 2):
                    nc.tensor.matmul(
                        out=ps,
                        lhsT=p.w_cproj[:, 2 * j : 2 * j + 2, 128 * m : 128 * (m + 1)],
                        rhs=g8[:, 2 * j : 2 * j + 2, :],
                        start=(j == 0), stop=(j == FT // 2 - 1), perf_mode=DR,
                    )
                nc.vector.affine_then_add(
                    out=p.x_t[m][:, qs].bitcast(F32R), in0=ps,
                    in1=p.x_t[m][:, qs], scale=D_CPROJ,
                    bias=p.b_cproj[:, m : m + 1],
                )
                # x_t[m][:, qs] is final -> store this chunk now
                nc.sync.dma_start(
                    out=io["yT"][128 * m : 128 * (m + 1), qs],
                    in_=p.x_t[m][:, qs],
                )


def emit_block(ctx, nc, tc, io, tag="", repeats=1):
    P = emit_prep(ctx, nc, tc, io, tag)
    for r in range(repeats):
        emit_body(nc, tc, io, P, tag + f"r{r}" if r else tag, reload_x=(r > 0))


def declare_io(nc):
    def inp(name, shape, dtype=F32):
        return nc.dram_tensor(name, shape, dtype, kind="ExternalInput").ap()

    io = {
        "xT": inp("xT", [C, T]),
        "wqkv8": inp("wqkv8", [128, CT, 3 * C], F8),
        "wproj8": inp("wproj8", [128, CT, C], F8),
        "wfc8": inp("wfc8", [128, CT, FF], F8),
        "wcproj8": inp("wcproj8", [128, FT, C], F8),
        "bqk": inp("bqk", [8, 128]),
        "bv16": inp("bv16", [1, C]),
        "bproj": inp("bproj", [CT, 128]),
        "bfc": inp("bfc", [FT, 128]),
        "bcproj": inp("bcproj", [CT, 128]),
        "ln1w": inp("ln1w", [CT, 128]),
        "ln1b": inp("ln1b", [CT, 128]),
        "ln2w": inp("ln2w", [CT, 128]),
        "ln2b": inp("ln2b", [CT, 128]),
        "ones_d": inp("ones_d", [128, 128]),
        "yT": nc.dram_tensor("yT", [C, T], F32, kind="ExternalOutput").ap(),
    }
    return io


def build(num_devices=N_CORES, repeats=1):
    nc = bacc.Bacc(
        "TRN2", target_bir_lowering=False, debug=False, num_devices=num_devices
    )
    # Pin Exp to the natural_log_exp table set (shared with Ln): the
    # default per-function set choice thrashes ACT_TABLE_LOADs between
    # exp_and_others and natural_log_exp on every LayerNorm.
    import concourse.hw_specs as _hws

    _tabs = _hws.get_activation_tables(nc.m.arch)
    for _name in ("exp_and_others", "exp_and_friends"):
        if _name in _tabs:
            _tabs[_name].clear()
    io = declare_io(nc)
    with tile.TileContext(nc) as tc, ExitStack() as ctx:
        emit_block(ctx, nc, tc, io, repeats=repeats)
    nc.compile()
    return nc


def _w8(w_t, scale):
    """[K, M] transposed weight -> DR-paired fp8 [128, K//128, M]."""
    f8 = mybir.dt.np(F8)
    k, m = w_t.shape
    return np.ascontiguousarray(
        (w_t * scale).reshape(k // 128, 128, m).transpose(1, 0, 2)
    ).astype(f8)


def host_inputs(x_b, attn_w, attn_b, proj_w, proj_b, fc_w, fc_b, cproj_w, cproj_b,
                ln1_w, ln1_b, ln2_w, ln2_b):
    """Per-core input dict for batch element x_b [T, C]."""
    f = np.float32
    return {
        "xT": np.ascontiguousarray(x_b.T, dtype=f),
        "wqkv8": _w8(attn_w.T.astype(f), SW),
        "wproj8": _w8(proj_w.T.astype(f), SW),
        "wfc8": _w8(fc_w.T.astype(f), SW),
        "wcproj8": _w8(cproj_w.T.astype(f), SWC),
        "bqk": np.ascontiguousarray(attn_b[: 2 * C].reshape(8, 128), dtype=f),
        "bv16": np.ascontiguousarray(
            (attn_b[2 * C :] * SV).reshape(1, C), dtype=f),
        "bproj": np.ascontiguousarray(proj_b.reshape(CT, 128), dtype=f),
        "bfc": np.ascontiguousarray(fc_b.reshape(FT, 128), dtype=f),
        "bcproj": np.ascontiguousarray(cproj_b.reshape(CT, 128), dtype=f),
        "ln1w": np.ascontiguousarray((ln1_w * SX).reshape(CT, 128), dtype=f),
        "ln1b": np.ascontiguousarray((ln1_b * SX).reshape(CT, 128), dtype=f),
        "ln2w": np.ascontiguousarray((ln2_w * SX).reshape(CT, 128), dtype=f),
        "ln2b": np.ascontiguousarray((ln2_b * SX).reshape(CT, 128), dtype=f),
        "ones_d": np.ones((128, 128), dtype=f),
    }


def unpack_output(result_map):
    """Map one core's output tensors to the [T, C] batch element."""
    return result_map["yT"].T


_CACHED_NC = None


def kernel(x, ln1_w, ln1_b, attn_w, attn_b, proj_w, proj_b,
           ln2_w, ln2_b, fc_w, fc_b, cproj_w, cproj_b):
    global _CACHED_NC
    x = np.asarray(x)
    B = x.shape[0]
    assert B == N_CORES and x.shape[1] == T and x.shape[2] == C
    if _CACHED_NC is None:
        _CACHED_NC = build()
    nc = _CACHED_NC
    args = [np.asarray(a, dtype=np.float32)
            for a in (attn_w, attn_b, proj_w, proj_b, fc_w, fc_b,
                      cproj_w, cproj_b, ln1_w, ln1_b, ln2_w, ln2_b)]
    (attn_w, attn_b, proj_w, proj_b, fc_w, fc_b,
     cproj_w, cproj_b, ln1_w, ln1_b, ln2_w, ln2_b) = args
    in_maps = [
        host_inputs(x[b], attn_w, attn_b, proj_w, proj_b, fc_w, fc_b,
                    cproj_w, cproj_b, ln1_w, ln1_b, ln2_w, ln2_b)
        for b in range(B)
    ]
    res = bass_utils.run_bass_kernel_spmd(
        nc, in_maps, core_ids=list(range(N_CORES))
    )
    out = np.empty((B, T, C), np.float32)
    for b in range(B):
        out[b] = unpack_output(res.results[b])
    return out


# revision 47
# speedup vs baseline: 1.0586x; 1.0586x over previous
"""Trainium2 Bass kernel for an nn.Block dense transformer layer.

Reference computation (per batch element b of 8):
    x = x + MHA(LN1(x));  x = x + MLP(LN2(x))
with T=1024 tokens, C=512 channels, H=16 heads (d=32), MLP hidden 2048,
new-gelu (tanh approx), softmax without causal mask.

Sharding: pure data parallelism - each of the 8 NeuronCores processes one
batch element. No collectives.

v2: fp8 (e4m3) DoubleRow matmuls for QKV/Proj/FC/CProj (K=256 per
instruction = 2x PE throughput), fp8 attention weights + values (plain
rate), per-head A.V + denominator as M=32 DR matmuls at partition base 0
with an aligned reciprocal-normalize and a partition-moving DMA into the
proj input layout. Activation-table schedule: natural_log_exp set for
LN1/attention/LN2, one switch to gelu_apprx_tanh. x is DMA'd before the
weights so LN1 starts immediately.

Scale scheme (validated vs reference in fp64/numpy, rel ~1.2e-2 < 2e-2):
  LN outs x16 (folded into ln w/b); weights x4096 (cproj x8192);
  exp out = 8*exp(s) (ln 8 folded into ACT bias); v8 = 16*v;
  dn ones = 0.5 so av/dn = 32*y; descales folded into PSUM-evac ops.
"""

import sys

if "/opt/trn_rl_repo" not in sys.path:
    sys.path.insert(0, "/opt/trn_rl_repo")

import math
from contextlib import ExitStack

import ml_dtypes
import numpy as np

import concourse.bass as bass
import concourse.mybir as mybir
import concourse.tile as tile
from concourse import bacc
from concourse import bass_utils

F32 = mybir.dt.float32
F32R = mybir.dt.float32r
BF16 = mybir.dt.bfloat16
F8 = mybir.dt.float8e4
AF = mybir.ActivationFunctionType
OP = mybir.AluOpType
DR = mybir.MatmulPerfMode.DoubleRow

N_CORES = 8
T = 1024  # tokens
C = 512  # channels
H = 16  # heads
D = 32  # head dim
FF = 2048  # mlp hidden
CT = C // 128  # channel partition tiles (4)
FT = FF // 128  # mlp hidden partition tiles (16)
NQ = T // 512  # token (query) 512-chunks (2)
G = H // 4  # head groups of 4 (4)
EPS = 1e-5
SCALE = 1.0 / math.sqrt(D)

SX = 16.0  # LN-output fp8 scale (folded into ln w/b host-side)
SW = 4096.0  # qkv/proj/fc weight scale
SWC = 8192.0  # cproj weight scale
SA = 8.0  # exp-output scale (ln SA folded into ACT bias)
SV = 16.0  # v fp8 scale
ONES_VAL = 0.5  # dn ones value -> av/dn = (SA*SV)/(SA*ONES_VAL) * y = 32*y
SY = SV / ONES_VAL  # 32
D_QKV = 1.0 / (SX * SW)  # 2^-16
D_V = SV / (SX * SW)  # 2^-12
D_PROJ = 1.0 / (SY * SW)  # 2^-17
D_FC = 1.0 / (SX * SW)  # 2^-16
D_CPROJ = 1.0 / (1.0 * SWC)  # 2^-13 (gelu out stored unscaled)
GELU_FUNC = AF.Gelu_apprx_tanh
# Schraudolph fast-exp on DVE for these key-tiles (offloads softmax exp
# work from ScalarE): bits = ACOEF*s + BCOEF, reinterpreted as fp32 =
# ~8*exp(s*SCALE) within +-4%. Disabled: the fp8 store needs a second
# full-rate pass on DVE/GpSimd, which costs more than ACT saves.
OFF_KTS = ()
SCH_A = float(2.0**23) * math.log2(math.e) * SCALE
SCH_B = float(2.0**23) * (127.0 + math.log2(SA) - 0.0579)


def r32(ap):
    return ap.bitcast(F32R)


class _NS:
    pass


def emit_prep(ctx, nc, tc, io, tag=""):
    """Persistent tiles + weight/const DMAs. x is loaded FIRST."""
    P = _NS()
    wpool = ctx.enter_context(tc.tile_pool(name="w" + tag, bufs=1))

    def single(shape, dtype, t):
        return wpool.tile(shape, dtype, tag=t, name=t)

    # ---- activations (persistent) ----
    x_t = [single([128, T], F32, f"xT{k}") for k in range(CT)]
    a8 = single([128, CT, T], F8, "a8")  # LN out *16, DR-paired layout
    q_t = [single([128, T], BF16, f"qT{g}") for g in range(G)]
    k_t = [single([128, T], BF16, f"kT{g}") for g in range(G)]
    # v8: [token, kt, head, 64] with cols 0:32 = 0.5 (dn-ones), cols
    # 32:64 = v*16, so one M=64 DR matmul yields dn rows 0:32 (reciprocal
    # reads PSUM directly, partition-aligned) + av rows 32:64.
    v8 = single([128, 8, H, 64], F8, "v8")
    av8 = single([128, G, T], F8, "av8")  # y*32, DR-paired for proj

    # x first so LN1 can start while weights stream in
    for k in range(CT):
        nc.sync.dma_start(
            out=x_t[k].bitcast(F32R),
            in_=io["xT"].bitcast(F32R)[128 * k : 128 * (k + 1), :],
        )

    # ---- fp8 weights (DR-paired layout [p, kt, out_features]) ----
    # qkv on the sync queue (needed first); the rest on the scalar-engine
    # queue so both DMA streams run in parallel with LN1 compute.
    w_qkv = single([128, CT, 3 * C], F8, "wqkv8")
    w_proj = single([128, CT, C], F8, "wproj8")
    w_fc = single([128, CT, FF], F8, "wfc8")
    w_cproj = single([128, FT, C], F8, "wcproj8")
    nc.sync.dma_start(out=w_qkv, in_=io["wqkv8"])
    nc.scalar.dma_start(out=w_fc, in_=io["wfc8"])
    nc.scalar.dma_start(out=w_cproj, in_=io["wcproj8"])
    nc.scalar.dma_start(out=w_proj, in_=io["wproj8"])

    # ---- bias / ln columns: tile[p, m] = vec[m*128 + p] ----
    def colmat(dram_ap, ntiles, t):
        tl = single([128, ntiles], F32, t)
        nc.sync.dma_start(out=tl, in_=dram_ap.transpose([1, 0]))
        return tl

    b_qk = colmat(io["bqk"], 8, "bqk")
    b_proj = colmat(io["bproj"], CT, "bproj")
    b_fc = colmat(io["bfc"], FT, "bfc")
    b_cproj = colmat(io["bcproj"], CT, "bcproj")
    ln1w = colmat(io["ln1w"], CT, "ln1w")  # pre-scaled *16 host-side
    ln1b = colmat(io["ln1b"], CT, "ln1b")
    ln2w = colmat(io["ln2w"], CT, "ln2w")
    ln2b = colmat(io["ln2b"], CT, "ln2b")

    # v bias broadcast (*16) to all partitions [128, C]
    bv_bc = single([128, C], F32, "bv_bc")
    nc.gpsimd.dma_start(
        out=bv_bc,
        in_=bass.AP(tensor=io["bv16"].tensor, offset=0, ap=[[0, 128], [1, C]]),
    )

    ones_f = single([128, 128], F32, "ones_f")
    nc.sync.dma_start(out=ones_f.bitcast(F32R), in_=io["ones_d"].bitcast(F32R))
    nc.vector.memset(v8, ONES_VAL)  # evac overwrites the v halves
    eps_t = single([128, 1], F32, "eps_t")
    nc.vector.memset(eps_t, EPS)
    ln_sa = single([128, 1], F32, "ln_sa")
    nc.vector.memset(ln_sa, math.log(SA))

    # rotating pools (SBUF)
    tmp = ctx.enter_context(tc.tile_pool(name="tmp" + tag, bufs=3))
    stat = ctx.enter_context(tc.tile_pool(name="stat" + tag, bufs=2))
    a2p = ctx.enter_context(tc.tile_pool(name="a2p" + tag, bufs=2))
    g8p = ctx.enter_context(tc.tile_pool(name="g8p" + tag, bufs=2))
    rcpp = ctx.enter_context(tc.tile_pool(name="rcp" + tag, bufs=3))
    y8p = ctx.enter_context(tc.tile_pool(name="y8p" + tag, bufs=3))
    dnp = ctx.enter_context(tc.tile_pool(name="dnp" + tag, bufs=2))
    schp = ctx.enter_context(tc.tile_pool(name="schp" + tag, bufs=2))

    for name in ("x_t", "a8", "q_t", "k_t", "v8", "av8", "w_qkv", "w_proj",
                 "w_fc", "w_cproj", "b_qk", "b_proj", "b_fc", "b_cproj",
                 "ln1w", "ln1b", "ln2w", "ln2b", "bv_bc", "ones_f",
                 "eps_t", "ln_sa", "tmp", "stat", "a2p", "g8p", "rcpp", "y8p",
                 "dnp", "schp"):
        setattr(P, name, locals()[name])
    return P


def emit_body(nc, tc, io, P, tag="", reload_x=False):
    p = P
    if reload_x:
        for k in range(CT):
            nc.sync.dma_start(
                out=p.x_t[k].bitcast(F32R),
                in_=io["xT"].bitcast(F32R)[128 * k : 128 * (k + 1), :],
            )

    # ---------------- LayerNorm (transposed domain) -> a8 fp8 -------------
    def layernorm(wcol, bcol, cols, psp, heavy=None):
        """LN over channel (partition) axis of x_t restricted to token
        range `cols`; writes (normalized*16) as fp8 into a8[:, k, cols].
        `heavy` picks the engine for the elementwise square/sub/mult ops
        (vector when latency-critical, gpsimd when DVE is the scarce one).
        """
        heavy = heavy or nc.vector
        ncols = cols.stop - cols.start
        musum = psp.tile([128, ncols], F32, tag="mm", name="ln_mu")
        sqsum = psp.tile([128, ncols], F32, tag="mm", name="ln_sq")
        for k in range(CT):
            sq = p.tmp.tile([128, ncols], F32, tag="sq", name="sq")
            heavy.tensor_tensor(
                out=sq.bitcast(F32R), in0=p.x_t[k][:, cols],
                in1=p.x_t[k][:, cols], op=OP.mult,
            )
            nc.tensor.matmul(
                out=musum, lhsT=r32(p.ones_f), rhs=r32(p.x_t[k][:, cols]),
                start=(k == 0), stop=(k == CT - 1),
            )
            nc.tensor.matmul(
                out=sqsum, lhsT=r32(p.ones_f), rhs=r32(sq),
                start=(k == 0), stop=(k == CT - 1),
            )
        mu = p.stat.tile([128, ncols], F32, tag="mu", name="mu")
        rstd = p.stat.tile([128, ncols], F32, tag="rstd", name="rstd")
        var = p.stat.tile([128, ncols], F32, tag="var", name="var")
        nc.vector.tensor_scalar_mul(out=mu, in0=musum, scalar1=1.0 / C)
        nc.vector.tensor_scalar_mul(out=var, in0=sqsum, scalar1=1.0 / C)
        nc.vector.tensor_tensor(out=rstd, in0=mu, in1=mu, op=OP.mult)
        nc.vector.tensor_tensor(out=var, in0=var, in1=rstd, op=OP.subtract)
        # rstd = exp(-0.5*ln(var+eps)) (stays on natural_log_exp table set)
        nc.scalar.activation(out=var, in_=var, func=AF.Ln, bias=p.eps_t, scale=1.0)
        nc.scalar.activation(out=rstd, in_=var, func=AF.Exp, bias=0.0, scale=-0.5)
        for k in range(CT):
            t1 = p.tmp.tile([128, ncols], F32, tag="t1", name="ln_t1")
            heavy.tensor_tensor(
                out=t1.bitcast(F32R), in0=p.x_t[k][:, cols], in1=mu, op=OP.subtract
            )
            heavy.tensor_tensor(out=t1.bitcast(F32R), in0=t1, in1=rstd, op=OP.mult)
            if k % 2:  # ScalarE is idle during the LN/QKV phases
                nc.scalar.activation(
                    out=p.a8[:, k, cols], in_=t1, func=AF.Identity,
                    bias=bcol[:, k : k + 1], scale=wcol[:, k : k + 1],
                )
            else:
                nc.vector.tensor_scalar(
                    out=p.a8[:, k, cols], in0=t1,
                    scalar1=wcol[:, k : k + 1], scalar2=bcol[:, k : k + 1],
                    op0=OP.mult, op1=OP.add,
                )

    # ======================= LN1 + QKV (DR fp8) ===========================
    with tc.tile_pool(name="ps1" + tag, bufs=4, space="PSUM") as pmm:
        # q^T, k^T: transposed out (feature on partitions), bf16 + bias.
        # nt-outer so chunk 1's LN overlaps chunk 0's QKV; evacuations
        # alternate DVE / ScalarE (idle here) by m parity.
        for nt in range(NQ):
            layernorm(p.ln1w, p.ln1b, slice(512 * nt, 512 * (nt + 1)), pmm,
                      heavy=nc.vector)
            for m in (0, 4, 1, 5, 2, 6, 3, 7):  # q/k pairs per head group
                dst = p.q_t[m] if m < 4 else p.k_t[m - 4]
                ps = pmm.tile([128, 512], F32, tag="mm", name="qk_ps")
                for j in range(2):
                    nc.tensor.matmul(
                        out=ps,
                        lhsT=p.w_qkv[:, 2 * j : 2 * j + 2, 128 * m : 128 * (m + 1)],
                        rhs=p.a8[:, 2 * j : 2 * j + 2, 512 * nt : 512 * (nt + 1)],
                        start=(j == 0), stop=(j == 1), perf_mode=DR,
                    )
                if m % 2:
                    nc.scalar.activation(
                        out=dst[:, 512 * nt : 512 * (nt + 1)], in_=ps,
                        func=AF.Identity, bias=p.b_qk[:, m : m + 1], scale=D_QKV,
                    )
                else:
                    nc.vector.tensor_scalar(
                        out=dst[:, 512 * nt : 512 * (nt + 1)], in0=ps,
                        scalar1=D_QKV, scalar2=p.b_qk[:, m : m + 1],
                        op0=OP.mult, op1=OP.add,
                    )
            # v natural layout [token, vfeat]: lhsT = a8 token-tile
            for t in range(4 * nt, 4 * nt + 4):
                ps = pmm.tile([128, C], F32, tag="mm", name="v_ps")
                for j in range(2):
                    nc.tensor.matmul(
                        out=ps,
                        lhsT=p.a8[:, 2 * j : 2 * j + 2, 128 * t : 128 * (t + 1)],
                        rhs=p.w_qkv[:, 2 * j : 2 * j + 2, 2 * C : 3 * C],
                        start=(j == 0), stop=(j == 1), perf_mode=DR,
                    )
                nc.vector.scalar_tensor_tensor(
                    out=p.v8[:, t, :, 32:64], in0=ps, scalar=D_V, in1=p.bv_bc,
                    op0=OP.mult, op1=OP.add,
                )

    # =========================== Attention ================================
    # per (qc, g): scores (bf16, 4-head row-packed) -> exp (fp8, *8) -> A2;
    # per head: av + dn as M=32 DR matmuls at partition 0, aligned
    # normalize, DMA into av8 row block.
    with tc.tile_pool(name="sc" + tag, bufs=1, space="PSUM") as scp, \
         tc.tile_pool(name="avdn" + tag, bufs=2, space="PSUM") as avp, \
         tc.tile_pool(name="mid" + tag, bufs=2, space="PSUM") as midp:
        sc_ctr = [0]

        def proj_ln2(qc):
            """proj + residual + LN2 for chunk qc; emitted mid-attention
            (runs on PE/DVE under the exp stream; Ln/Exp share the loaded
            natural_log_exp set)."""
            qs = slice(512 * qc, 512 * (qc + 1))
            for m in range(CT):
                ps = midp.tile([128, 512], F32, tag="mm", name="proj_ps")
                for j in range(2):
                    nc.tensor.matmul(
                        out=ps,
                        lhsT=p.w_proj[:, 2 * j : 2 * j + 2, 128 * m : 128 * (m + 1)],
                        rhs=p.av8[:, 2 * j : 2 * j + 2, qs],
                        start=(j == 0), stop=(j == 1), perf_mode=DR,
                    )
                nc.vector.affine_then_add(
                    out=p.x_t[m][:, qs].bitcast(F32R), in0=ps,
                    in1=p.x_t[m][:, qs], scale=D_PROJ,
                    bias=p.b_proj[:, m : m + 1],
                )
            layernorm(p.ln2w, p.ln2b, qs, midp, heavy=nc.gpsimd)
        # Software pipeline: group g's A.V matmuls are interleaved two-per-
        # kt-iteration into group g+1's scores stream, so ScalarE's exp
        # pipeline never stalls behind a burst of AV work on the PE.
        av_state = {}

        def emit_av_step(a2_prev, g_prev, qs_prev, step):
            h, j = divmod(step, 4)
            hg = 4 * g_prev + h
            if j == 0:
                av_state[h] = avp.tile([64, 512], F32, tag="av", name="av_ps")
            av_ps = av_state[h]
            nc.tensor.matmul(
                out=av_ps,
                lhsT=p.v8[:, 2 * j : 2 * j + 2, hg, :],
                rhs=a2_prev[h // 2][
                    :, 2 * j : 2 * j + 2, 512 * (h % 2) : 512 * (h % 2) + 512,
                ],
                start=(j == 0), stop=(j == 3), perf_mode=DR,
            )
            if j == 3:
                # dn at rows 0:31 -> reciprocal straight off PSUM (aligned);
                # shift rcp to rows 32:63 by DMA; aligned normalize-multiply
                # at rows 32:63; placement DMA into av8's row block.
                rcp = p.rcpp.tile([32, 512], F32, tag="rcp", name="rcp")
                nc.vector.reciprocal_approx_fast(out=rcp, in_=av_ps[0:32, :])
                rcps = p.dnp.tile([64, 512], F32, tag="rcps", name="rcps")
                nc.gpsimd.dma_start(out=rcps[32:64, :], in_=rcp)
                y8s = p.y8p.tile([64, 512], F8, tag="y8", name="y8s")
                nc.vector.tensor_tensor(
                    out=y8s[32:64, :], in0=av_ps[32:64, :],
                    in1=rcps[32:64, :], op=OP.mult,
                )
                nc.gpsimd.dma_start(
                    out=p.av8[32 * h : 32 * h + 32, g_prev, qs_prev],
                    in_=y8s[32:64, :],
                )

        prev = None
        for qc in range(NQ):
            qs = slice(512 * qc, 512 * (qc + 1))
            for g in range(G):
                a2 = [p.a2p.tile([128, 8, 1024], F8, tag=f"a2_{i}", name="a2")
                      for i in range(2)]
                for half in range(2):
                    for kt in range(4):
                        ktg = 4 * half + kt
                        sc = []
                        for i in range(2):
                            t2 = sc_ctr[0] % 2
                            sc_ctr[0] += 1
                            sc.append(scp.tile([128, 1024], F32,
                                               tag=f"sc{t2}", name="sc"))
                        for c in range(4):
                            pr = slice(32 * c, 32 * (c + 1))
                            nc.tensor.matmul(
                                out=sc[c // 2][:, 512 * (c % 2) : 512 * (c % 2 + 1)],
                                lhsT=p.k_t[g][pr, 128 * ktg : 128 * (ktg + 1)],
                                rhs=p.q_t[g][pr, qs],
                                start=True, stop=True,
                                tile_position=(32 * c, 0),
                            )
                        for i in range(2):
                            if ktg in OFF_KTS:
                                ti = p.schp.tile(
                                    [128, 1024], mybir.dt.int32,
                                    tag="sch", name="sch",
                                )
                                nc.vector.tensor_scalar(
                                    out=ti, in0=sc[i], scalar1=SCH_A,
                                    scalar2=SCH_B, op0=OP.mult, op1=OP.add,
                                )
                                nc.gpsimd.tensor_copy(
                                    a2[i][:, ktg, :], ti.bitcast(F32)
                                )
                            else:
                                nc.scalar.activation(
                                    out=a2[i][:, ktg, :], in_=sc[i], func=AF.Exp,
                                    bias=p.ln_sa, scale=SCALE,
                                )
                if prev is not None:
                    for _ in range(16):
                        emit_av_step(*prev)
                        prev = (prev[0], prev[1], prev[2], prev[3] + 1)
                    if qc == 1 and g == 1:
                        # chunk 0's AV fully drained during (qc1, g0)
                        proj_ln2(0)
                prev = (a2, g, qs, 0)
        # drain the last group's AV work
        for _ in range(16):
            emit_av_step(*prev)
            prev = (prev[0], prev[1], prev[2], prev[3] + 1)
        proj_ln2(1)

    # ======================== MLP (DR fp8) ================================
    with tc.tile_pool(name="ps2" + tag, bufs=4, space="PSUM") as pmm:
        for qc in range(NQ):
            qs = slice(512 * qc, 512 * (qc + 1))
            g8 = p.g8p.tile([128, FT, 512], F8, tag="g8", name="g8")
            for m in range(FT):
                ps = pmm.tile([128, 512], F32, tag="mm", name="fc_ps")
                for j in range(2):
                    nc.tensor.matmul(
                        out=ps,
                        lhsT=p.w_fc[:, 2 * j : 2 * j + 2, 128 * m : 128 * (m + 1)],
                        rhs=p.a8[:, 2 * j : 2 * j + 2, qs],
                        start=(j == 0), stop=(j == 1), perf_mode=DR,
                    )
                nc.scalar.activation(
                    out=g8[:, m, :], in_=ps, func=GELU_FUNC,
                    bias=p.b_fc[:, m : m + 1], scale=D_FC,
                )
            for m in range(CT):
                ps = pmm.tile([128, 512], F32, tag="mm", name="cproj_ps")
                for j in range(FT // 2):
                    nc.tensor.matmul(
                        out=ps,
                        lhsT=p.w_cproj[:, 2 * j : 2 * j + 2, 128 * m : 128 * (m + 1)],
                        rhs=g8[:, 2 * j : 2 * j + 2, :],
                        start=(j == 0), stop=(j == FT // 2 - 1), perf_mode=DR,
                    )
                nc.vector.affine_then_add(
                    out=p.x_t[m][:, qs].bitcast(F32R), in0=ps,
                    in1=p.x_t[m][:, qs], scale=D_CPROJ,
                    bias=p.b_cproj[:, m : m + 1],
                )
                # x_t[m][:, qs] is final -> store this chunk now
                nc.sync.dma_start(
                    out=io["yT"][128 * m : 128 * (m + 1), qs],
                    in_=p.x_t[m][:, qs],
                )


def emit_block(ctx, nc, tc, io, tag="", repeats=1):
    P = emit_prep(ctx, nc, tc, io, tag)
    for r in range(repeats):
        emit_body(nc, tc, io, P, tag + f"r{r}" if r else tag, reload_x=(r > 0))


def declare_io(nc):
    def inp(name, shape, dtype=F32):
        return nc.dram_tensor(name, shape, dtype, kind="ExternalInput").ap()

    io = {
        "xT": inp("xT", [C, T]),
        "wqkv8": inp("wqkv8", [128, CT, 3 * C], F8),
        "wproj8": inp("wproj8", [128, CT, C], F8),
        "wfc8": inp("wfc8", [128, CT, FF], F8),
        "wcproj8": inp("wcproj8", [128, FT, C], F8),
        "bqk": inp("bqk", [8, 128]),
        "bv16": inp("bv16", [1, C]),
        "bproj": inp("bproj", [CT, 128]),
        "bfc": inp("bfc", [FT, 128]),
        "bcproj": inp("bcproj", [CT, 128]),
        "ln1w": inp("ln1w", [CT, 128]),
        "ln1b": inp("ln1b", [CT, 128]),
        "ln2w": inp("ln2w", [CT, 128]),
        "ln2b": inp("ln2b", [CT, 128]),
        "ones_d": inp("ones_d", [128, 128]),
        "yT": nc.dram_tensor("yT", [C, T], F32, kind="ExternalOutput").ap(),
    }
    return io


def build(num_devices=N_CORES, repeats=1):
    nc = bacc.Bacc(
        "TRN2", target_bir_lowering=False, debug=False, num_devices=num_devices
    )
    # Pin Exp to the natural_log_exp table set (shared with Ln): the
    # default per-function set choice thrashes ACT_TABLE_LOADs between
    # exp_and_others and natural_log_exp on every LayerNorm.
    import concourse.hw_specs as _hws

    _tabs = _hws.get_activation_tables(nc.m.arch)
    for _name in ("exp_and_others", "exp_and_friends"):
        if _name in _tabs:
            _tabs[_name].clear()
    io = declare_io(nc)
    with tile.TileContext(nc) as tc, ExitStack() as ctx:
        emit_block(ctx, nc, tc, io, repeats=repeats)
    nc.compile()
    return nc


def _w8(w_t, scale):
    """[K, M] transposed weight -> DR-paired fp8 [128, K//128, M]."""
    f8 = mybir.dt.np(F8)
    k, m = w_t.shape
    return np.ascontiguousarray(
        (w_t * scale).reshape(k // 128, 128, m).transpose(1, 0, 2)
    ).astype(f8)


def host_inputs(x_b, attn_w, attn_b, proj_w, proj_b, fc_w, fc_b, cproj_w, cproj_b,
                ln1_w, ln1_b, ln2_w, ln2_b):
    """Per-core input dict for batch element x_b [T, C]."""
    f = np.float32
    return {
        "xT": np.ascontiguousarray(x_b.T, dtype=f),
        "wqkv8": _w8(attn_w.T.astype(f), SW),
        "wproj8": _w8(proj_w.T.astype(f), SW),
        "wfc8": _w8(fc_w.T.astype(f), SW),
        "wcproj8": _w8(cproj_w.T.astype(f), SWC),
        "bqk": np.ascontiguousarray(attn_b[: 2 * C].reshape(8, 128), dtype=f),
        "bv16": np.ascontiguousarray(
            (attn_b[2 * C :] * SV).reshape(1, C), dtype=f),
        "bproj": np.ascontiguousarray(proj_b.reshape(CT, 128), dtype=f),
        "bfc": np.ascontiguousarray(fc_b.reshape(FT, 128), dtype=f),
        "bcproj": np.ascontiguousarray(cproj_b.reshape(CT, 128), dtype=f),
        "ln1w": np.ascontiguousarray((ln1_w * SX).reshape(CT, 128), dtype=f),
        "ln1b": np.ascontiguousarray((ln1_b * SX).reshape(CT, 128), dtype=f),
        "ln2w": np.ascontiguousarray((ln2_w * SX).reshape(CT, 128), dtype=f),
        "ln2b": np.ascontiguousarray((ln2_b * SX).reshape(CT, 128), dtype=f),
        "ones_d": np.ones((128, 128), dtype=f),
    }


def unpack_output(result_map):
    """Map one core's output tensors to the [T, C] batch element."""
    return result_map["yT"].T


_CACHED_NC = None


def kernel(x, ln1_w, ln1_b, attn_w, attn_b, proj_w, proj_b,
           ln2_w, ln2_b, fc_w, fc_b, cproj_w, cproj_b):
    global _CACHED_NC
    x = np.asarray(x)
    B = x.shape[0]
    assert B == N_CORES and x.shape[1] == T and x.shape[2] == C
    if _CACHED_NC is None:
        _CACHED_NC = build()
    nc = _CACHED_NC
    args = [np.asarray(a, dtype=np.float32)
            for a in (attn_w, attn_b, proj_w, proj_b, fc_w, fc_b,
                      cproj_w, cproj_b, ln1_w, ln1_b, ln2_w, ln2_b)]
    (attn_w, attn_b, proj_w, proj_b, fc_w, fc_b,
     cproj_w, cproj_b, ln1_w, ln1_b, ln2_w, ln2_b) = args
    in_maps = [
        host_inputs(x[b], attn_w, attn_b, proj_w, proj_b, fc_w, fc_b,
                    cproj_w, cproj_b, ln1_w, ln1_b, ln2_w, ln2_b)
        for b in range(B)
    ]
    res = bass_utils.run_bass_kernel_spmd(
        nc, in_maps, core_ids=list(range(N_CORES))
    )
    out = np.empty((B, T, C), np.float32)
    for b in range(B):
        out[b] = unpack_output(res.results[b])
    return out


# revision 49
# speedup vs baseline: 1.0668x; 1.0077x over previous
"""Trainium2 Bass kernel for an nn.Block dense transformer layer.

Reference computation (per batch element b of 8):
    x = x + MHA(LN1(x));  x = x + MLP(LN2(x))
with T=1024 tokens, C=512 channels, H=16 heads (d=32), MLP hidden 2048,
new-gelu (tanh approx), softmax without causal mask.

Sharding: pure data parallelism - each of the 8 NeuronCores processes one
batch element. No collectives.

v2: fp8 (e4m3) DoubleRow matmuls for QKV/Proj/FC/CProj (K=256 per
instruction = 2x PE throughput), fp8 attention weights + values (plain
rate), per-head A.V + denominator as M=32 DR matmuls at partition base 0
with an aligned reciprocal-normalize and a partition-moving DMA into the
proj input layout. Activation-table schedule: natural_log_exp set for
LN1/attention/LN2, one switch to gelu_apprx_tanh. x is DMA'd before the
weights so LN1 starts immediately.

Scale scheme (validated vs reference in fp64/numpy, rel ~1.2e-2 < 2e-2):
  LN outs x16 (folded into ln w/b); weights x4096 (cproj x8192);
  exp out = 8*exp(s) (ln 8 folded into ACT bias); v8 = 16*v;
  dn ones = 0.5 so av/dn = 32*y; descales folded into PSUM-evac ops.
"""

import sys

if "/opt/trn_rl_repo" not in sys.path:
    sys.path.insert(0, "/opt/trn_rl_repo")

import math
from contextlib import ExitStack

import ml_dtypes
import numpy as np

import concourse.bass as bass
import concourse.mybir as mybir
import concourse.tile as tile
from concourse import bacc
from concourse import bass_utils

F32 = mybir.dt.float32
F32R = mybir.dt.float32r
BF16 = mybir.dt.bfloat16
F8 = mybir.dt.float8e4
AF = mybir.ActivationFunctionType
OP = mybir.AluOpType
DR = mybir.MatmulPerfMode.DoubleRow

N_CORES = 8
T = 1024  # tokens
C = 512  # channels
H = 16  # heads
D = 32  # head dim
FF = 2048  # mlp hidden
CT = C // 128  # channel partition tiles (4)
FT = FF // 128  # mlp hidden partition tiles (16)
NQ = T // 512  # token (query) 512-chunks (2)
G = H // 4  # head groups of 4 (4)
EPS = 1e-5
SCALE = 1.0 / math.sqrt(D)

SX = 16.0  # LN-output fp8 scale (folded into ln w/b host-side)
SW = 4096.0  # qkv/proj/fc weight scale
SWC = 8192.0  # cproj weight scale
SA = 8.0  # exp-output scale (ln SA folded into ACT bias)
SV = 16.0  # v fp8 scale
ONES_VAL = 0.5  # dn ones value -> av/dn = (SA*SV)/(SA*ONES_VAL) * y = 32*y
SY = SV / ONES_VAL  # 32
D_QKV = 1.0 / (SX * SW)  # 2^-16
D_V = SV / (SX * SW)  # 2^-12
D_PROJ = 1.0 / (SY * SW)  # 2^-17
D_FC = 1.0 / (SX * SW)  # 2^-16
D_CPROJ = 1.0 / (1.0 * SWC)  # 2^-13 (gelu out stored unscaled)
GELU_FUNC = AF.Gelu_apprx_tanh
# Schraudolph fast-exp on DVE for these key-tiles (offloads softmax exp
# work from ScalarE): bits = ACOEF*s + BCOEF, reinterpreted as fp32 =
# ~8*exp(s*SCALE) within +-4%. Disabled: the fp8 store needs a second
# full-rate pass on DVE/GpSimd, which costs more than ACT saves.
OFF_KTS = ()
SCH_A = float(2.0**23) * math.log2(math.e) * SCALE
SCH_B = float(2.0**23) * (127.0 + math.log2(SA) - 0.0579)


def r32(ap):
    return ap.bitcast(F32R)


class _NS:
    pass


def emit_prep(ctx, nc, tc, io, tag=""):
    """Persistent tiles + weight/const DMAs. x is loaded FIRST."""
    P = _NS()
    wpool = ctx.enter_context(tc.tile_pool(name="w" + tag, bufs=1))

    def single(shape, dtype, t):
        return wpool.tile(shape, dtype, tag=t, name=t)

    # ---- activations (persistent) ----
    x_t = [single([128, T], F32, f"xT{k}") for k in range(CT)]
    a8 = single([128, CT, T], F8, "a8")  # LN out *16, DR-paired layout
    q_t = [single([128, T], BF16, f"qT{g}") for g in range(G)]
    k_t = [single([128, T], BF16, f"kT{g}") for g in range(G)]
    # v8: [token, kt, head, 64] with cols 0:32 = 0.5 (dn-ones), cols
    # 32:64 = v*16, so one M=64 DR matmul yields dn rows 0:32 (reciprocal
    # reads PSUM directly, partition-aligned) + av rows 32:64.
    v8 = single([128, 8, H, 64], F8, "v8")
    av8 = single([128, G, T], F8, "av8")  # y*32, DR-paired for proj

    # x first so LN1 can start while weights stream in; chunked by token
    # half so LN1(chunk 0) unblocks after the first four half-transfers
    for nt in range(NQ):
        for k in range(CT):
            nc.sync.dma_start(
                out=x_t[k].bitcast(F32R)[:, 512 * nt : 512 * (nt + 1)],
                in_=io["xT"].bitcast(F32R)[
                    128 * k : 128 * (k + 1), 512 * nt : 512 * (nt + 1)
                ],
            )

    # ---- fp8 weights (DR-paired layout [p, kt, out_features]) ----
    # qkv on the sync queue (needed first); the rest on the scalar-engine
    # queue so both DMA streams run in parallel with LN1 compute.
    w_qkv = single([128, CT, 3 * C], F8, "wqkv8")
    w_proj = single([128, CT, C], F8, "wproj8")
    w_fc = single([128, CT, FF], F8, "wfc8")
    w_cproj = single([128, FT, C], F8, "wcproj8")
    nc.sync.dma_start(out=w_qkv, in_=io["wqkv8"])
    nc.scalar.dma_start(out=w_fc, in_=io["wfc8"])
    nc.scalar.dma_start(out=w_cproj, in_=io["wcproj8"])
    nc.scalar.dma_start(out=w_proj, in_=io["wproj8"])

    # ---- bias / ln columns: tile[p, m] = vec[m*128 + p] ----
    def colmat(dram_ap, ntiles, t):
        tl = single([128, ntiles], F32, t)
        nc.sync.dma_start(out=tl, in_=dram_ap.transpose([1, 0]))
        return tl

    b_qk = colmat(io["bqk"], 8, "bqk")
    b_proj = colmat(io["bproj"], CT, "bproj")
    b_fc = colmat(io["bfc"], FT, "bfc")
    b_cproj = colmat(io["bcproj"], CT, "bcproj")
    ln1w = colmat(io["ln1w"], CT, "ln1w")  # pre-scaled *16 host-side
    ln1b = colmat(io["ln1b"], CT, "ln1b")
    ln2w = colmat(io["ln2w"], CT, "ln2w")
    ln2b = colmat(io["ln2b"], CT, "ln2b")

    # v bias broadcast (*16) to all partitions [128, C]
    bv_bc = single([128, C], F32, "bv_bc")
    nc.gpsimd.dma_start(
        out=bv_bc,
        in_=bass.AP(tensor=io["bv16"].tensor, offset=0, ap=[[0, 128], [1, C]]),
    )

    ones_f = single([128, 128], F32, "ones_f")
    nc.sync.dma_start(out=ones_f.bitcast(F32R), in_=io["ones_d"].bitcast(F32R))
    nc.vector.memset(v8, ONES_VAL)  # evac overwrites the v halves
    eps_t = single([128, 1], F32, "eps_t")
    nc.vector.memset(eps_t, EPS)
    ln_sa = single([128, 1], F32, "ln_sa")
    nc.vector.memset(ln_sa, math.log(SA))

    # rotating pools (SBUF)
    tmp = ctx.enter_context(tc.tile_pool(name="tmp" + tag, bufs=3))
    stat = ctx.enter_context(tc.tile_pool(name="stat" + tag, bufs=2))
    a2p = ctx.enter_context(tc.tile_pool(name="a2p" + tag, bufs=2))
    g8p = ctx.enter_context(tc.tile_pool(name="g8p" + tag, bufs=2))
    rcpp = ctx.enter_context(tc.tile_pool(name="rcp" + tag, bufs=3))
    y8p = ctx.enter_context(tc.tile_pool(name="y8p" + tag, bufs=3))
    dnp = ctx.enter_context(tc.tile_pool(name="dnp" + tag, bufs=2))
    schp = ctx.enter_context(tc.tile_pool(name="schp" + tag, bufs=2))

    for name in ("x_t", "a8", "q_t", "k_t", "v8", "av8", "w_qkv", "w_proj",
                 "w_fc", "w_cproj", "b_qk", "b_proj", "b_fc", "b_cproj",
                 "ln1w", "ln1b", "ln2w", "ln2b", "bv_bc", "ones_f",
                 "eps_t", "ln_sa", "tmp", "stat", "a2p", "g8p", "rcpp", "y8p",
                 "dnp", "schp"):
        setattr(P, name, locals()[name])
    return P


def emit_body(nc, tc, io, P, tag="", reload_x=False):
    p = P
    if reload_x:
        for k in range(CT):
            nc.sync.dma_start(
                out=p.x_t[k].bitcast(F32R),
                in_=io["xT"].bitcast(F32R)[128 * k : 128 * (k + 1), :],
            )

    # ---------------- LayerNorm (transposed domain) -> a8 fp8 -------------
    def layernorm(wcol, bcol, cols, psp, heavy=None):
        """LN over channel (partition) axis of x_t restricted to token
        range `cols`; writes (normalized*16) as fp8 into a8[:, k, cols].
        `heavy` picks the engine for the elementwise square/sub/mult ops
        (vector when latency-critical, gpsimd when DVE is the scarce one).
        """
        heavy = heavy or nc.vector
        ncols = cols.stop - cols.start
        musum = psp.tile([128, ncols], F32, tag="mm", name="ln_mu")
        sqsum = psp.tile([128, ncols], F32, tag="mm", name="ln_sq")
        for k in range(CT):
            sq = p.tmp.tile([128, ncols], F32, tag="sq", name="sq")
            heavy.tensor_tensor(
                out=sq.bitcast(F32R), in0=p.x_t[k][:, cols],
                in1=p.x_t[k][:, cols], op=OP.mult,
            )
            nc.tensor.matmul(
                out=musum, lhsT=r32(p.ones_f), rhs=r32(p.x_t[k][:, cols]),
                start=(k == 0), stop=(k == CT - 1),
            )
            nc.tensor.matmul(
                out=sqsum, lhsT=r32(p.ones_f), rhs=r32(sq),
                start=(k == 0), stop=(k == CT - 1),
            )
        mu = p.stat.tile([128, ncols], F32, tag="mu", name="mu")
        rstd = p.stat.tile([128, ncols], F32, tag="rstd", name="rstd")
        var = p.stat.tile([128, ncols], F32, tag="var", name="var")
        nc.vector.tensor_scalar_mul(out=mu, in0=musum, scalar1=1.0 / C)
        nc.vector.tensor_scalar_mul(out=var, in0=sqsum, scalar1=1.0 / C)
        nc.vector.tensor_tensor(out=rstd, in0=mu, in1=mu, op=OP.mult)
        nc.vector.tensor_tensor(out=var, in0=var, in1=rstd, op=OP.subtract)
        # rstd = exp(-0.5*ln(var+eps)) (stays on natural_log_exp table set)
        nc.scalar.activation(out=var, in_=var, func=AF.Ln, bias=p.eps_t, scale=1.0)
        nc.scalar.activation(out=rstd, in_=var, func=AF.Exp, bias=0.0, scale=-0.5)
        for k in range(CT):
            t1 = p.tmp.tile([128, ncols], F32, tag="t1", name="ln_t1")
            heavy.tensor_tensor(
                out=t1.bitcast(F32R), in0=p.x_t[k][:, cols], in1=mu, op=OP.subtract
            )
            heavy.tensor_tensor(out=t1.bitcast(F32R), in0=t1, in1=rstd, op=OP.mult)
            if k % 2:  # ScalarE is idle during the LN/QKV phases
                nc.scalar.activation(
                    out=p.a8[:, k, cols], in_=t1, func=AF.Identity,
                    bias=bcol[:, k : k + 1], scale=wcol[:, k : k + 1],
                )
            else:
                nc.vector.tensor_scalar(
                    out=p.a8[:, k, cols], in0=t1,
                    scalar1=wcol[:, k : k + 1], scalar2=bcol[:, k : k + 1],
                    op0=OP.mult, op1=OP.add,
                )

    # ======================= LN1 + QKV (DR fp8) ===========================
    with tc.tile_pool(name="ps1" + tag, bufs=4, space="PSUM") as pmm:
        # q^T, k^T: transposed out (feature on partitions), bf16 + bias.
        # nt-outer so chunk 1's LN overlaps chunk 0's QKV; evacuations
        # alternate DVE / ScalarE (idle here) by m parity.
        for nt in range(NQ):
            layernorm(p.ln1w, p.ln1b, slice(512 * nt, 512 * (nt + 1)), pmm,
                      heavy=nc.vector)
            # nt0: q/k pairs per head group; nt1: k tiles first so head
            # group g's scores (which need k over all T) unblock earliest.
            m_order = (0, 4, 1, 5, 2, 6, 3, 7) if nt == 0 else (4, 0, 5, 1, 6, 2, 7, 3)
            for m in m_order:
                dst = p.q_t[m] if m < 4 else p.k_t[m - 4]
                ps = pmm.tile([128, 512], F32, tag="mm", name="qk_ps")
                for j in range(2):
                    nc.tensor.matmul(
                        out=ps,
                        lhsT=p.w_qkv[:, 2 * j : 2 * j + 2, 128 * m : 128 * (m + 1)],
                        rhs=p.a8[:, 2 * j : 2 * j + 2, 512 * nt : 512 * (nt + 1)],
                        start=(j == 0), stop=(j == 1), perf_mode=DR,
                    )
                if m % 2:
                    nc.scalar.activation(
                        out=dst[:, 512 * nt : 512 * (nt + 1)], in_=ps,
                        func=AF.Identity, bias=p.b_qk[:, m : m + 1], scale=D_QKV,
                    )
                else:
                    nc.vector.tensor_scalar(
                        out=dst[:, 512 * nt : 512 * (nt + 1)], in0=ps,
                        scalar1=D_QKV, scalar2=p.b_qk[:, m : m + 1],
                        op0=OP.mult, op1=OP.add,
                    )
            # v natural layout [token, vfeat]: lhsT = a8 token-tile
            for t in range(4 * nt, 4 * nt + 4):
                ps = pmm.tile([128, C], F32, tag="mm", name="v_ps")
                for j in range(2):
                    nc.tensor.matmul(
                        out=ps,
                        lhsT=p.a8[:, 2 * j : 2 * j + 2, 128 * t : 128 * (t + 1)],
                        rhs=p.w_qkv[:, 2 * j : 2 * j + 2, 2 * C : 3 * C],
                        start=(j == 0), stop=(j == 1), perf_mode=DR,
                    )
                nc.vector.scalar_tensor_tensor(
                    out=p.v8[:, t, :, 32:64], in0=ps, scalar=D_V, in1=p.bv_bc,
                    op0=OP.mult, op1=OP.add,
                )

    # =========================== Attention ================================
    # per (qc, g): scores (bf16, 4-head row-packed) -> exp (fp8, *8) -> A2;
    # per head: av + dn as M=32 DR matmuls at partition 0, aligned
    # normalize, DMA into av8 row block.
    with tc.tile_pool(name="sc" + tag, bufs=1, space="PSUM") as scp, \
         tc.tile_pool(name="avdn" + tag, bufs=2, space="PSUM") as avp, \
         tc.tile_pool(name="mid" + tag, bufs=2, space="PSUM") as midp:
        sc_ctr = [0]

        def proj_ln2(qc):
            """proj + residual + LN2 for chunk qc; emitted mid-attention
            (runs on PE/DVE under the exp stream; Ln/Exp share the loaded
            natural_log_exp set)."""
            qs = slice(512 * qc, 512 * (qc + 1))
            for m in range(CT):
                ps = midp.tile([128, 512], F32, tag="mm", name="proj_ps")
                for j in range(2):
                    nc.tensor.matmul(
                        out=ps,
                        lhsT=p.w_proj[:, 2 * j : 2 * j + 2, 128 * m : 128 * (m + 1)],
                        rhs=p.av8[:, 2 * j : 2 * j + 2, qs],
                        start=(j == 0), stop=(j == 1), perf_mode=DR,
                    )
                nc.vector.affine_then_add(
                    out=p.x_t[m][:, qs].bitcast(F32R), in0=ps,
                    in1=p.x_t[m][:, qs], scale=D_PROJ,
                    bias=p.b_proj[:, m : m + 1],
                )
            layernorm(p.ln2w, p.ln2b, qs, midp, heavy=nc.gpsimd)
        # Software pipeline: group g's A.V matmuls are interleaved two-per-
        # kt-iteration into group g+1's scores stream, so ScalarE's exp
        # pipeline never stalls behind a burst of AV work on the PE.
        av_state = {}

        def emit_av_step(a2_prev, g_prev, qs_prev, step):
            h, j = divmod(step, 4)
            hg = 4 * g_prev + h
            if j == 0:
                av_state[h] = avp.tile([64, 512], F32, tag="av", name="av_ps")
            av_ps = av_state[h]
            nc.tensor.matmul(
                out=av_ps,
                lhsT=p.v8[:, 2 * j : 2 * j + 2, hg, :],
                rhs=a2_prev[h // 2][
                    :, 2 * j : 2 * j + 2, 512 * (h % 2) : 512 * (h % 2) + 512,
                ],
                start=(j == 0), stop=(j == 3), perf_mode=DR,
            )
            if j == 3:
                # dn at rows 0:31 -> reciprocal straight off PSUM (aligned);
                # shift rcp to rows 32:63 by DMA; aligned normalize-multiply
                # at rows 32:63; placement DMA into av8's row block.
                rcp = p.rcpp.tile([32, 512], F32, tag="rcp", name="rcp")
                nc.vector.reciprocal_approx_fast(out=rcp, in_=av_ps[0:32, :])
                rcps = p.dnp.tile([64, 512], F32, tag="rcps", name="rcps")
                nc.gpsimd.dma_start(out=rcps[32:64, :], in_=rcp)
                y8s = p.y8p.tile([64, 512], F8, tag="y8", name="y8s")
                nc.vector.tensor_tensor(
                    out=y8s[32:64, :], in0=av_ps[32:64, :],
                    in1=rcps[32:64, :], op=OP.mult,
                )
                nc.gpsimd.dma_start(
                    out=p.av8[32 * h : 32 * h + 32, g_prev, qs_prev],
                    in_=y8s[32:64, :],
                )

        prev = None
        for qc in range(NQ):
            qs = slice(512 * qc, 512 * (qc + 1))
            for g in range(G):
                a2 = [p.a2p.tile([128, 8, 1024], F8, tag=f"a2_{i}", name="a2")
                      for i in range(2)]
                for half in range(2):
                    for kt in range(4):
                        ktg = 4 * half + kt
                        sc = []
                        for i in range(2):
                            t2 = sc_ctr[0] % 2
                            sc_ctr[0] += 1
                            sc.append(scp.tile([128, 1024], F32,
                                               tag=f"sc{t2}", name="sc"))
                        for c in range(4):
                            pr = slice(32 * c, 32 * (c + 1))
                            nc.tensor.matmul(
                                out=sc[c // 2][:, 512 * (c % 2) : 512 * (c % 2 + 1)],
                                lhsT=p.k_t[g][pr, 128 * ktg : 128 * (ktg + 1)],
                                rhs=p.q_t[g][pr, qs],
                                start=True, stop=True,
                                tile_position=(32 * c, 0),
                            )
                        for i in range(2):
                            if ktg in OFF_KTS:
                                ti = p.schp.tile(
                                    [128, 1024], mybir.dt.int32,
                                    tag="sch", name="sch",
                                )
                                nc.vector.tensor_scalar(
                                    out=ti, in0=sc[i], scalar1=SCH_A,
                                    scalar2=SCH_B, op0=OP.mult, op1=OP.add,
                                )
                                nc.gpsimd.tensor_copy(
                                    a2[i][:, ktg, :], ti.bitcast(F32)
                                )
                            else:
                                nc.scalar.activation(
                                    out=a2[i][:, ktg, :], in_=sc[i], func=AF.Exp,
                                    bias=p.ln_sa, scale=SCALE,
                                )
                if prev is not None:
                    for _ in range(16):
                        emit_av_step(*prev)
                        prev = (prev[0], prev[1], prev[2], prev[3] + 1)
                    if qc == 1 and g == 1:
                        # chunk 0's AV fully drained during (qc1, g0)
                        proj_ln2(0)
                prev = (a2, g, qs, 0)
        # drain the last group's AV work
        for _ in range(16):
            emit_av_step(*prev)
            prev = (prev[0], prev[1], prev[2], prev[3] + 1)
        proj_ln2(1)

    # ======================== MLP (DR fp8) ================================
    with tc.tile_pool(name="ps2" + tag, bufs=4, space="PSUM") as pmm:
        for qc in range(NQ):
            qs = slice(512 * qc, 512 * (qc + 1))
            g8 = p.g8p.tile([128, FT, 512], F8, tag="g8", name="g8")
            for m in range(FT):
                ps = pmm.tile([128, 512], F32, tag="mm", name="fc_ps")
                for j in range(2):
                    nc.tensor.matmul(
                        out=ps,
                        lhsT=p.w_fc[:, 2 * j : 2 * j + 2, 128 * m : 128 * (m + 1)],
                        rhs=p.a8[:, 2 * j : 2 * j + 2, qs],
                        start=(j == 0), stop=(j == 1), perf_mode=DR,
                    )
                nc.scalar.activation(
                    out=g8[:, m, :], in_=ps, func=GELU_FUNC,
                    bias=p.b_fc[:, m : m + 1], scale=D_FC,
                )
            for m in range(CT):
                ps = pmm.tile([128, 512], F32, tag="mm", name="cproj_ps")
                for j in range(FT // 2):
                    nc.tensor.matmul(
                        out=ps,
                        lhsT=p.w_cproj[:, 2 * j : 2 * j + 2, 128 * m : 128 * (m + 1)],
                        rhs=g8[:, 2 * j : 2 * j + 2, :],
                        start=(j == 0), stop=(j == FT // 2 - 1), perf_mode=DR,
                    )
                nc.vector.affine_then_add(
                    out=p.x_t[m][:, qs].bitcast(F32R), in0=ps,
                    in1=p.x_t[m][:, qs], scale=D_CPROJ,
                    bias=p.b_cproj[:, m : m + 1],
                )
                # x_t[m][:, qs] is final -> store this chunk now
                nc.sync.dma_start(
                    out=io["yT"][128 * m : 128 * (m + 1), qs],
                    in_=p.x_t[m][:, qs],
                )


def emit_block(ctx, nc, tc, io, tag="", repeats=1):
    P = emit_prep(ctx, nc, tc, io, tag)
    for r in range(repeats):
        emit_body(nc, tc, io, P, tag + f"r{r}" if r else tag, reload_x=(r > 0))


def declare_io(nc):
    def inp(name, shape, dtype=F32):
        return nc.dram_tensor(name, shape, dtype, kind="ExternalInput").ap()

    io = {
        "xT": inp("xT", [C, T]),
        "wqkv8": inp("wqkv8", [128, CT, 3 * C], F8),
        "wproj8": inp("wproj8", [128, CT, C], F8),
        "wfc8": inp("wfc8", [128, CT, FF], F8),
        "wcproj8": inp("wcproj8", [128, FT, C], F8),
        "bqk": inp("bqk", [8, 128]),
        "bv16": inp("bv16", [1, C]),
        "bproj": inp("bproj", [CT, 128]),
        "bfc": inp("bfc", [FT, 128]),
        "bcproj": inp("bcproj", [CT, 128]),
        "ln1w": inp("ln1w", [CT, 128]),
        "ln1b": inp("ln1b", [CT, 128]),
        "ln2w": inp("ln2w", [CT, 128]),
        "ln2b": inp("ln2b", [CT, 128]),
        "ones_d": inp("ones_d", [128, 128]),
        "yT": nc.dram_tensor("yT", [C, T], F32, kind="ExternalOutput").ap(),
    }
    return io


def build(num_devices=N_CORES, repeats=1):
    nc = bacc.Bacc(
        "TRN2", target_bir_lowering=False, debug=False, num_devices=num_devices
    )
    # Pin Exp to the natural_log_exp table set (shared with Ln): the
    # default per-function set choice thrashes ACT_TABLE_LOADs between
    # exp_and_others and natural_log_exp on every LayerNorm.
    import concourse.hw_specs as _hws

    _tabs = _hws.get_activation_tables(nc.m.arch)
    for _name in ("exp_and_others", "exp_and_friends"):
        if _name in _tabs:
            _tabs[_name].clear()
    io = declare_io(nc)
    with tile.TileContext(nc) as tc, ExitStack() as ctx:
        emit_block(ctx, nc, tc, io, repeats=repeats)
    nc.compile()
    return nc


def _w8(w_t, scale):
    """[K, M] transposed weight -> DR-paired fp8 [128, K//128, M]."""
    f8 = mybir.dt.np(F8)
    k, m = w_t.shape
    return np.ascontiguousarray(
        (w_t * scale).reshape(k // 128, 128, m).transpose(1, 0, 2)
    ).astype(f8)


def host_inputs(x_b, attn_w, attn_b, proj_w, proj_b, fc_w, fc_b, cproj_w, cproj_b,
                ln1_w, ln1_b, ln2_w, ln2_b):
    """Per-core input dict for batch element x_b [T, C]."""
    f = np.float32
    return {
        "xT": np.ascontiguousarray(x_b.T, dtype=f),
        "wqkv8": _w8(attn_w.T.astype(f), SW),
        "wproj8": _w8(proj_w.T.astype(f), SW),
        "wfc8": _w8(fc_w.T.astype(f), SW),
        "wcproj8": _w8(cproj_w.T.astype(f), SWC),
        "bqk": np.ascontiguousarray(attn_b[: 2 * C].reshape(8, 128), dtype=f),
        "bv16": np.ascontiguousarray(
            (attn_b[2 * C :] * SV).reshape(1, C), dtype=f),
        "bproj": np.ascontiguousarray(proj_b.reshape(CT, 128), dtype=f),
        "bfc": np.ascontiguousarray(fc_b.reshape(FT, 128), dtype=f),
        "bcproj": np.ascontiguousarray(cproj_b.reshape(CT, 128), dtype=f),
        "ln1w": np.ascontiguousarray((ln1_w * SX).reshape(CT, 128), dtype=f),
        "ln1b": np.ascontiguousarray((ln1_b * SX).reshape(CT, 128), dtype=f),
        "ln2w": np.ascontiguousarray((ln2_w * SX).reshape(CT, 128), dtype=f),
        "ln2b": np.ascontiguousarray((ln2_b * SX).reshape(CT, 128), dtype=f),
        "ones_d": np.ones((128, 128), dtype=f),
    }


def unpack_output(result_map):
    """Map one core's output tensors to the [T, C] batch element."""
    return result_map["yT"].T


_CACHED_NC = None


def kernel(x, ln1_w, ln1_b, attn_w, attn_b, proj_w, proj_b,
           ln2_w, ln2_b, fc_w, fc_b, cproj_w, cproj_b):
    global _CACHED_NC
    x = np.asarray(x)
    B = x.shape[0]
    assert B == N_CORES and x.shape[1] == T and x.shape[2] == C
    if _CACHED_NC is None:
        _CACHED_NC = build()
    nc = _CACHED_NC
    args = [np.asarray(a, dtype=np.float32)
            for a in (attn_w, attn_b, proj_w, proj_b, fc_w, fc_b,
                      cproj_w, cproj_b, ln1_w, ln1_b, ln2_w, ln2_b)]
    (attn_w, attn_b, proj_w, proj_b, fc_w, fc_b,
     cproj_w, cproj_b, ln1_w, ln1_b, ln2_w, ln2_b) = args
    in_maps = [
        host_inputs(x[b], attn_w, attn_b, proj_w, proj_b, fc_w, fc_b,
                    cproj_w, cproj_b, ln1_w, ln1_b, ln2_w, ln2_b)
        for b in range(B)
    ]
    res = bass_utils.run_bass_kernel_spmd(
        nc, in_maps, core_ids=list(range(N_CORES))
    )
    out = np.empty((B, T, C), np.float32)
    for b in range(B):
        out[b] = unpack_output(res.results[b])
    return out


# revision 52
# speedup vs baseline: 1.1060x; 1.0367x over previous
"""Trainium2 Bass kernel for an nn.Block dense transformer layer.

Reference computation (per batch element b of 8):
    x = x + MHA(LN1(x));  x = x + MLP(LN2(x))
with T=1024 tokens, C=512 channels, H=16 heads (d=32), MLP hidden 2048,
new-gelu (tanh approx), softmax without causal mask.

Sharding: pure data parallelism - each of the 8 NeuronCores processes one
batch element. No collectives.

v2: fp8 (e4m3) DoubleRow matmuls for QKV/Proj/FC/CProj (K=256 per
instruction = 2x PE throughput), fp8 attention weights + values (plain
rate), per-head A.V + denominator as M=32 DR matmuls at partition base 0
with an aligned reciprocal-normalize and a partition-moving DMA into the
proj input layout. Activation-table schedule: natural_log_exp set for
LN1/attention/LN2, one switch to gelu_apprx_tanh. x is DMA'd before the
weights so LN1 starts immediately.

Scale scheme (validated vs reference in fp64/numpy, rel ~1.2e-2 < 2e-2):
  LN outs x16 (folded into ln w/b); weights x4096 (cproj x8192);
  exp out = 8*exp(s) (ln 8 folded into ACT bias); v8 = 16*v;
  dn ones = 0.5 so av/dn = 32*y; descales folded into PSUM-evac ops.
"""

import sys

if "/opt/trn_rl_repo" not in sys.path:
    sys.path.insert(0, "/opt/trn_rl_repo")

import math
from contextlib import ExitStack

import ml_dtypes
import numpy as np

import concourse.bass as bass
import concourse.mybir as mybir
import concourse.tile as tile
from concourse import bacc
from concourse import bass_utils

F32 = mybir.dt.float32
F32R = mybir.dt.float32r
BF16 = mybir.dt.bfloat16
F8 = mybir.dt.float8e4
AF = mybir.ActivationFunctionType
OP = mybir.AluOpType
DR = mybir.MatmulPerfMode.DoubleRow

N_CORES = 8
T = 1024  # tokens
C = 512  # channels
H = 16  # heads
D = 32  # head dim
FF = 2048  # mlp hidden
CT = C // 128  # channel partition tiles (4)
FT = FF // 128  # mlp hidden partition tiles (16)
NQ = T // 512  # token (query) 512-chunks (2)
G = H // 4  # head groups of 4 (4)
EPS = 1e-5
SCALE = 1.0 / math.sqrt(D)

SX = 16.0  # LN-output fp8 scale (folded into ln w/b host-side)
SW = 4096.0  # qkv/proj/fc weight scale
SWC = 8192.0  # cproj weight scale
SA = 8.0  # exp-output scale (ln SA folded into ACT bias)
SV = 16.0  # v fp8 scale
ONES_VAL = 0.5  # dn ones value -> av/dn = (SA*SV)/(SA*ONES_VAL) * y = 32*y
SY = SV / ONES_VAL  # 32
D_QKV = 1.0 / (SX * SW)  # 2^-16
D_V = SV / (SX * SW)  # 2^-12
D_PROJ = 1.0 / (SY * SW)  # 2^-17
D_FC = 1.0 / (SX * SW)  # 2^-16
D_CPROJ = 1.0 / (1.0 * SWC)  # 2^-13 (gelu out stored unscaled)
GELU_FUNC = AF.Gelu_apprx_tanh
# Schraudolph fast-exp on DVE for these key-tiles (offloads softmax exp
# work from ScalarE): bits = ACOEF*s + BCOEF, reinterpreted as fp32 =
# ~8*exp(s*SCALE) within +-4%. Disabled: the fp8 store needs a second
# full-rate pass on DVE/GpSimd, which costs more than ACT saves.
OFF_KTS = ()
SCH_A = float(2.0**23) * math.log2(math.e) * SCALE
SCH_B = float(2.0**23) * (127.0 + math.log2(SA) - 0.0579)


def r32(ap):
    return ap.bitcast(F32R)


class _NS:
    pass


def emit_prep(ctx, nc, tc, io, tag=""):
    """Persistent tiles + weight/const DMAs. x is loaded FIRST."""
    P = _NS()
    wpool = ctx.enter_context(tc.tile_pool(name="w" + tag, bufs=1))

    def single(shape, dtype, t):
        return wpool.tile(shape, dtype, tag=t, name=t)

    # ---- activations (persistent) ----
    x_t = [single([128, T], F32, f"xT{k}") for k in range(CT)]
    a8 = single([128, CT, T], F8, "a8")  # LN out *16, DR-paired layout
    q_t = [single([128, T], BF16, f"qT{g}") for g in range(G)]
    k_t = [single([128, T], BF16, f"kT{g}") for g in range(G)]
    # v8: [token, kt, head, 64] with cols 0:32 = 0.5 (dn-ones), cols
    # 32:64 = v*16, so one M=64 DR matmul yields dn rows 0:32 (reciprocal
    # reads PSUM directly, partition-aligned) + av rows 32:64.
    v8 = single([128, 8, H, 64], F8, "v8")
    av8 = single([128, G, T], F8, "av8")  # y*32, DR-paired for proj

    # x first so LN1 can start while weights stream in; chunked by token
    # half so LN1(chunk 0) unblocks after the first four half-transfers
    for nt in range(NQ):
        for k in range(CT):
            nc.sync.dma_start(
                out=x_t[k].bitcast(F32R)[:, 512 * nt : 512 * (nt + 1)],
                in_=io["xT"].bitcast(F32R)[
                    128 * k : 128 * (k + 1), 512 * nt : 512 * (nt + 1)
                ],
            )

    # ---- fp8 weights (DR-paired layout [p, kt, out_features]) ----
    # qkv on the sync queue (needed first); the rest on the scalar-engine
    # queue so both DMA streams run in parallel with LN1 compute.
    w_qkv = single([128, CT, 3 * C], F8, "wqkv8")
    w_proj = single([128, CT, C], F8, "wproj8")
    w_fc = single([128, CT, FF], F8, "wfc8")
    w_cproj = single([128, FT, C], F8, "wcproj8")
    nc.sync.dma_start(out=w_qkv, in_=io["wqkv8"])
    nc.scalar.dma_start(out=w_fc, in_=io["wfc8"])
    nc.scalar.dma_start(out=w_cproj, in_=io["wcproj8"])
    nc.scalar.dma_start(out=w_proj, in_=io["wproj8"])

    # ---- bias / ln columns: tile[p, m] = vec[m*128 + p] ----
    def colmat(dram_ap, ntiles, t):
        tl = single([128, ntiles], F32, t)
        nc.sync.dma_start(out=tl, in_=dram_ap.transpose([1, 0]))
        return tl

    b_qk = colmat(io["bqk"], 8, "bqk")
    b_proj = colmat(io["bproj"], CT, "bproj")
    b_fc = colmat(io["bfc"], FT, "bfc")
    b_cproj = colmat(io["bcproj"], CT, "bcproj")
    ln1w = colmat(io["ln1w"], CT, "ln1w")  # pre-scaled *16 host-side
    ln1b = colmat(io["ln1b"], CT, "ln1b")
    ln2w = colmat(io["ln2w"], CT, "ln2w")
    ln2b = colmat(io["ln2b"], CT, "ln2b")

    # v bias broadcast (*16) to all partitions [128, C]
    bv_bc = single([128, C], F32, "bv_bc")
    nc.gpsimd.dma_start(
        out=bv_bc,
        in_=bass.AP(tensor=io["bv16"].tensor, offset=0, ap=[[0, 128], [1, C]]),
    )

    ones_f = single([128, 128], F32, "ones_f")
    nc.sync.dma_start(out=ones_f.bitcast(F32R), in_=io["ones_d"].bitcast(F32R))
    nc.vector.memset(v8, ONES_VAL)  # evac overwrites the v halves
    eps_t = single([128, 1], F32, "eps_t")
    nc.vector.memset(eps_t, EPS)
    ln_sa = single([128, 1], F32, "ln_sa")
    nc.vector.memset(ln_sa, math.log(SA))

    # rotating pools (SBUF)
    tmp = ctx.enter_context(tc.tile_pool(name="tmp" + tag, bufs=3))
    stat = ctx.enter_context(tc.tile_pool(name="stat" + tag, bufs=2))
    a2p = ctx.enter_context(tc.tile_pool(name="a2p" + tag, bufs=2))
    g8p = ctx.enter_context(tc.tile_pool(name="g8p" + tag, bufs=2))
    rcpp = ctx.enter_context(tc.tile_pool(name="rcp" + tag, bufs=3))
    y8p = ctx.enter_context(tc.tile_pool(name="y8p" + tag, bufs=3))
    dnp = ctx.enter_context(tc.tile_pool(name="dnp" + tag, bufs=2))
    schp = ctx.enter_context(tc.tile_pool(name="schp" + tag, bufs=2))

    for name in ("x_t", "a8", "q_t", "k_t", "v8", "av8", "w_qkv", "w_proj",
                 "w_fc", "w_cproj", "b_qk", "b_proj", "b_fc", "b_cproj",
                 "ln1w", "ln1b", "ln2w", "ln2b", "bv_bc", "ones_f",
                 "eps_t", "ln_sa", "tmp", "stat", "a2p", "g8p", "rcpp", "y8p",
                 "dnp", "schp"):
        setattr(P, name, locals()[name])
    return P


def emit_body(nc, tc, io, P, tag="", reload_x=False):
    p = P
    if reload_x:
        for k in range(CT):
            nc.sync.dma_start(
                out=p.x_t[k].bitcast(F32R),
                in_=io["xT"].bitcast(F32R)[128 * k : 128 * (k + 1), :],
            )

    # ---------------- LayerNorm (transposed domain) -> a8 fp8 -------------
    def layernorm(wcol, bcol, cols, psp, heavy=None):
        """LN over channel (partition) axis of x_t restricted to token
        range `cols`; writes (normalized*16) as fp8 into a8[:, k, cols].
        `heavy` picks the engine for the elementwise square/sub/mult ops
        (vector when latency-critical, gpsimd when DVE is the scarce one).
        """
        heavy = heavy or nc.vector
        ncols = cols.stop - cols.start
        musum = psp.tile([128, ncols], F32, tag="mm", name="ln_mu")
        sqsum = psp.tile([128, ncols], F32, tag="mm", name="ln_sq")
        for k in range(CT):
            sq = p.tmp.tile([128, ncols], F32, tag="sq", name="sq")
            heavy.tensor_tensor(
                out=sq.bitcast(F32R), in0=p.x_t[k][:, cols],
                in1=p.x_t[k][:, cols], op=OP.mult,
            )
            nc.tensor.matmul(
                out=musum, lhsT=r32(p.ones_f), rhs=r32(p.x_t[k][:, cols]),
                start=(k == 0), stop=(k == CT - 1),
            )
            nc.tensor.matmul(
                out=sqsum, lhsT=r32(p.ones_f), rhs=r32(sq),
                start=(k == 0), stop=(k == CT - 1),
            )
        mu = p.stat.tile([128, ncols], F32, tag="mu", name="mu")
        rstd = p.stat.tile([128, ncols], F32, tag="rstd", name="rstd")
        var = p.stat.tile([128, ncols], F32, tag="var", name="var")
        nc.vector.tensor_scalar_mul(out=mu, in0=musum, scalar1=1.0 / C)
        nc.vector.tensor_scalar_mul(out=var, in0=sqsum, scalar1=1.0 / C)
        nc.vector.tensor_tensor(out=rstd, in0=mu, in1=mu, op=OP.mult)
        nc.vector.tensor_tensor(out=var, in0=var, in1=rstd, op=OP.subtract)
        # rstd = exp(-0.5*ln(var+eps)) (stays on natural_log_exp table set)
        nc.scalar.activation(out=var, in_=var, func=AF.Ln, bias=p.eps_t, scale=1.0)
        nc.scalar.activation(out=rstd, in_=var, func=AF.Exp, bias=0.0, scale=-0.5)
        for k in range(CT):
            t1 = p.tmp.tile([128, ncols], F32, tag="t1", name="ln_t1")
            heavy.tensor_tensor(
                out=t1.bitcast(F32R), in0=p.x_t[k][:, cols], in1=mu, op=OP.subtract
            )
            heavy.tensor_tensor(out=t1.bitcast(F32R), in0=t1, in1=rstd, op=OP.mult)
            nc.scalar.activation(
                out=p.a8[:, k, cols], in_=t1, func=AF.Identity,
                bias=bcol[:, k : k + 1], scale=wcol[:, k : k + 1],
            )

    # ======================= LN1 + QKV (DR fp8) ===========================
    with tc.tile_pool(name="ps1" + tag, bufs=4, space="PSUM") as pmm:
        # q^T, k^T: transposed out (feature on partitions), bf16 + bias.
        # nt-outer so chunk 1's LN overlaps chunk 0's QKV; evacuations
        # alternate DVE / ScalarE (idle here) by m parity.
        for nt in range(NQ):
            layernorm(p.ln1w, p.ln1b, slice(512 * nt, 512 * (nt + 1)), pmm,
                      heavy=nc.vector)
            # nt0: q/k pairs per head group; nt1: k tiles first so head
            # group g's scores (which need k over all T) unblock earliest.
            m_order = (0, 4, 1, 5, 2, 6, 3, 7) if nt == 0 else (4, 0, 5, 1, 6, 2, 7, 3)
            for m in m_order:
                dst = p.q_t[m] if m < 4 else p.k_t[m - 4]
                ps = pmm.tile([128, 512], F32, tag="mm", name="qk_ps")
                for j in range(2):
                    nc.tensor.matmul(
                        out=ps,
                        lhsT=p.w_qkv[:, 2 * j : 2 * j + 2, 128 * m : 128 * (m + 1)],
                        rhs=p.a8[:, 2 * j : 2 * j + 2, 512 * nt : 512 * (nt + 1)],
                        start=(j == 0), stop=(j == 1), perf_mode=DR,
                    )
                nc.scalar.activation(
                    out=dst[:, 512 * nt : 512 * (nt + 1)], in_=ps,
                    func=AF.Identity, bias=p.b_qk[:, m : m + 1], scale=D_QKV,
                )
            # v natural layout [token, vfeat]: lhsT = a8 token-tile
            for t in range(4 * nt, 4 * nt + 4):
                ps = pmm.tile([128, C], F32, tag="mm", name="v_ps")
                for j in range(2):
                    nc.tensor.matmul(
                        out=ps,
                        lhsT=p.a8[:, 2 * j : 2 * j + 2, 128 * t : 128 * (t + 1)],
                        rhs=p.w_qkv[:, 2 * j : 2 * j + 2, 2 * C : 3 * C],
                        start=(j == 0), stop=(j == 1), perf_mode=DR,
                    )
                nc.vector.scalar_tensor_tensor(
                    out=p.v8[:, t, :, 32:64], in0=ps, scalar=D_V, in1=p.bv_bc,
                    op0=OP.mult, op1=OP.add,
                )

    # =========================== Attention ================================
    # per (qc, g): scores (bf16, 4-head row-packed) -> exp (fp8, *8) -> A2;
    # per head: av + dn as M=32 DR matmuls at partition 0, aligned
    # normalize, DMA into av8 row block.
    with tc.tile_pool(name="sc" + tag, bufs=1, space="PSUM") as scp, \
         tc.tile_pool(name="avdn" + tag, bufs=2, space="PSUM") as avp, \
         tc.tile_pool(name="mid" + tag, bufs=2, space="PSUM") as midp:
        sc_ctr = [0]

        def proj_ln2(qc):
            """proj + residual + LN2 for chunk qc; emitted mid-attention
            (runs on PE/DVE under the exp stream; Ln/Exp share the loaded
            natural_log_exp set)."""
            qs = slice(512 * qc, 512 * (qc + 1))
            for m in range(CT):
                ps = midp.tile([128, 512], F32, tag="mm", name="proj_ps")
                for j in range(2):
                    nc.tensor.matmul(
                        out=ps,
                        lhsT=p.w_proj[:, 2 * j : 2 * j + 2, 128 * m : 128 * (m + 1)],
                        rhs=p.av8[:, 2 * j : 2 * j + 2, qs],
                        start=(j == 0), stop=(j == 1), perf_mode=DR,
                    )
                nc.vector.affine_then_add(
                    out=p.x_t[m][:, qs].bitcast(F32R), in0=ps,
                    in1=p.x_t[m][:, qs], scale=D_PROJ,
                    bias=p.b_proj[:, m : m + 1],
                )
            layernorm(p.ln2w, p.ln2b, qs, midp, heavy=nc.vector)
        # Software pipeline: group g's A.V matmuls are interleaved two-per-
        # kt-iteration into group g+1's scores stream, so ScalarE's exp
        # pipeline never stalls behind a burst of AV work on the PE.
        av_state = {}

        def emit_av_step(a2_prev, g_prev, qs_prev, step):
            h, j = divmod(step, 4)
            hg = 4 * g_prev + h
            if j == 0:
                av_state[h] = avp.tile([64, 512], F32, tag="av", name="av_ps")
            av_ps = av_state[h]
            nc.tensor.matmul(
                out=av_ps,
                lhsT=p.v8[:, 2 * j : 2 * j + 2, hg, :],
                rhs=a2_prev[h // 2][
                    :, 2 * j : 2 * j + 2, 512 * (h % 2) : 512 * (h % 2) + 512,
                ],
                start=(j == 0), stop=(j == 3), perf_mode=DR,
            )
            if j == 3:
                # dn at rows 0:31 -> reciprocal straight off PSUM (aligned);
                # shift rcp to rows 32:63 by DMA; aligned normalize-multiply
                # at rows 32:63; placement DMA into av8's row block.
                rcp = p.rcpp.tile([32, 512], F32, tag="rcp", name="rcp")
                nc.vector.reciprocal_approx_fast(out=rcp, in_=av_ps[0:32, :])
                rcps = p.dnp.tile([64, 512], F32, tag="rcps", name="rcps")
                nc.gpsimd.dma_start(out=rcps[32:64, :], in_=rcp)
                y8s = p.y8p.tile([64, 512], F8, tag="y8", name="y8s")
                nc.vector.tensor_tensor(
                    out=y8s[32:64, :], in0=av_ps[32:64, :],
                    in1=rcps[32:64, :], op=OP.mult,
                )
                nc.gpsimd.dma_start(
                    out=p.av8[32 * h : 32 * h + 32, g_prev, qs_prev],
                    in_=y8s[32:64, :],
                )

        prev = None
        for qc in range(NQ):
            qs = slice(512 * qc, 512 * (qc + 1))
            for g in range(G):
                a2 = [p.a2p.tile([128, 8, 1024], F8, tag=f"a2_{i}", name="a2")
                      for i in range(2)]
                for half in range(2):
                    for kt in range(4):
                        ktg = 4 * half + kt
                        sc = []
                        for i in range(2):
                            t2 = sc_ctr[0] % 2
                            sc_ctr[0] += 1
                            sc.append(scp.tile([128, 1024], F32,
                                               tag=f"sc{t2}", name="sc"))
                        for c in range(4):
                            pr = slice(32 * c, 32 * (c + 1))
                            nc.tensor.matmul(
                                out=sc[c // 2][:, 512 * (c % 2) : 512 * (c % 2 + 1)],
                                lhsT=p.k_t[g][pr, 128 * ktg : 128 * (ktg + 1)],
                                rhs=p.q_t[g][pr, qs],
                                start=True, stop=True,
                                tile_position=(32 * c, 0),
                            )
                        for i in range(2):
                            if ktg in OFF_KTS:
                                ti = p.schp.tile(
                                    [128, 1024], mybir.dt.int32,
                                    tag="sch", name="sch",
                                )
                                nc.vector.tensor_scalar(
                                    out=ti, in0=sc[i], scalar1=SCH_A,
                                    scalar2=SCH_B, op0=OP.mult, op1=OP.add,
                                )
                                nc.gpsimd.tensor_copy(
                                    a2[i][:, ktg, :], ti.bitcast(F32)
                                )
                            else:
                                nc.scalar.activation(
                                    out=a2[i][:, ktg, :], in_=sc[i], func=AF.Exp,
                                    bias=p.ln_sa, scale=SCALE,
                                )
                if prev is not None:
                    for _ in range(16):
                        emit_av_step(*prev)
                        prev = (prev[0], prev[1], prev[2], prev[3] + 1)
                    if qc == 1 and g == 1:
                        # chunk 0's AV fully drained during (qc1, g0)
                        proj_ln2(0)
                prev = (a2, g, qs, 0)
        # drain the last group's AV work
        for _ in range(16):
            emit_av_step(*prev)
            prev = (prev[0], prev[1], prev[2], prev[3] + 1)
        proj_ln2(1)

    # ======================== MLP (DR fp8) ================================
    with tc.tile_pool(name="ps2" + tag, bufs=4, space="PSUM") as pmm:
        for qc in range(NQ):
            qs = slice(512 * qc, 512 * (qc + 1))
            g8 = p.g8p.tile([128, FT, 512], F8, tag="g8", name="g8")
            for m in range(FT):
                ps = pmm.tile([128, 512], F32, tag="mm", name="fc_ps")
                for j in range(2):
                    nc.tensor.matmul(
                        out=ps,
                        lhsT=p.w_fc[:, 2 * j : 2 * j + 2, 128 * m : 128 * (m + 1)],
                        rhs=p.a8[:, 2 * j : 2 * j + 2, qs],
                        start=(j == 0), stop=(j == 1), perf_mode=DR,
                    )
                nc.scalar.activation(
                    out=g8[:, m, :], in_=ps, func=GELU_FUNC,
                    bias=p.b_fc[:, m : m + 1], scale=D_FC,
                )
            for m in range(CT):
                ps = pmm.tile([128, 512], F32, tag="mm", name="cproj_ps")
                for j in range(FT // 2):
                    nc.tensor.matmul(
                        out=ps,
                        lhsT=p.w_cproj[:, 2 * j : 2 * j + 2, 128 * m : 128 * (m + 1)],
                        rhs=g8[:, 2 * j : 2 * j + 2, :],
                        start=(j == 0), stop=(j == FT // 2 - 1), perf_mode=DR,
                    )
                nc.vector.affine_then_add(
                    out=p.x_t[m][:, qs].bitcast(F32R), in0=ps,
                    in1=p.x_t[m][:, qs], scale=D_CPROJ,
                    bias=p.b_cproj[:, m : m + 1],
                )
                # x_t[m][:, qs] is final -> store this chunk now
                nc.sync.dma_start(
                    out=io["yT"][128 * m : 128 * (m + 1), qs],
                    in_=p.x_t[m][:, qs],
                )


def emit_block(ctx, nc, tc, io, tag="", repeats=1):
    P = emit_prep(ctx, nc, tc, io, tag)
    for r in range(repeats):
        emit_body(nc, tc, io, P, tag + f"r{r}" if r else tag, reload_x=(r > 0))


def declare_io(nc):
    def inp(name, shape, dtype=F32):
        return nc.dram_tensor(name, shape, dtype, kind="ExternalInput").ap()

    io = {
        "xT": inp("xT", [C, T]),
        "wqkv8": inp("wqkv8", [128, CT, 3 * C], F8),
        "wproj8": inp("wproj8", [128, CT, C], F8),
        "wfc8": inp("wfc8", [128, CT, FF], F8),
        "wcproj8": inp("wcproj8", [128, FT, C], F8),
        "bqk": inp("bqk", [8, 128]),
        "bv16": inp("bv16", [1, C]),
        "bproj": inp("bproj", [CT, 128]),
        "bfc": inp("bfc", [FT, 128]),
        "bcproj": inp("bcproj", [CT, 128]),
        "ln1w": inp("ln1w", [CT, 128]),
        "ln1b": inp("ln1b", [CT, 128]),
        "ln2w": inp("ln2w", [CT, 128]),
        "ln2b": inp("ln2b", [CT, 128]),
        "ones_d": inp("ones_d", [128, 128]),
        "yT": nc.dram_tensor("yT", [C, T], F32, kind="ExternalOutput").ap(),
    }
    return io


def build(num_devices=N_CORES, repeats=1):
    nc = bacc.Bacc(
        "TRN2", target_bir_lowering=False, debug=False, num_devices=num_devices
    )
    # Pin Exp to the natural_log_exp table set (shared with Ln): the
    # default per-function set choice thrashes ACT_TABLE_LOADs between
    # exp_and_others and natural_log_exp on every LayerNorm.
    import concourse.hw_specs as _hws

    _tabs = _hws.get_activation_tables(nc.m.arch)
    for _name in ("exp_and_others", "exp_and_friends"):
        if _name in _tabs:
            _tabs[_name].clear()
    io = declare_io(nc)
    with tile.TileContext(nc) as tc, ExitStack() as ctx:
        emit_block(ctx, nc, tc, io, repeats=repeats)
    nc.compile()
    return nc


def _w8(w_t, scale):
    """[K, M] transposed weight -> DR-paired fp8 [128, K//128, M]."""
    f8 = mybir.dt.np(F8)
    k, m = w_t.shape
    return np.ascontiguousarray(
        (w_t * scale).reshape(k // 128, 128, m).transpose(1, 0, 2)
    ).astype(f8)


def host_inputs(x_b, attn_w, attn_b, proj_w, proj_b, fc_w, fc_b, cproj_w, cproj_b,
                ln1_w, ln1_b, ln2_w, ln2_b):
    """Per-core input dict for batch element x_b [T, C]."""
    f = np.float32
    return {
        "xT": np.ascontiguousarray(x_b.T, dtype=f),
        "wqkv8": _w8(attn_w.T.astype(f), SW),
        "wproj8": _w8(proj_w.T.astype(f), SW),
        "wfc8": _w8(fc_w.T.astype(f), SW),
        "wcproj8": _w8(cproj_w.T.astype(f), SWC),
        "bqk": np.ascontiguousarray(attn_b[: 2 * C].reshape(8, 128), dtype=f),
        "bv16": np.ascontiguousarray(
            (attn_b[2 * C :] * SV).reshape(1, C), dtype=f),
        "bproj": np.ascontiguousarray(proj_b.reshape(CT, 128), dtype=f),
        "bfc": np.ascontiguousarray(fc_b.reshape(FT, 128), dtype=f),
        "bcproj": np.ascontiguousarray(cproj_b.reshape(CT, 128), dtype=f),
        "ln1w": np.ascontiguousarray((ln1_w * SX).reshape(CT, 128), dtype=f),
        "ln1b": np.ascontiguousarray((ln1_b * SX).reshape(CT, 128), dtype=f),
        "ln2w": np.ascontiguousarray((ln2_w * SX).reshape(CT, 128), dtype=f),
        "ln2b": np.ascontiguousarray((ln2_b * SX).reshape(CT, 128), dtype=f),
        "ones_d": np.ones((128, 128), dtype=f),
    }


def unpack_output(result_map):
    """Map one core's output tensors to the [T, C] batch element."""
    return result_map["yT"].T


_CACHED_NC = None


def kernel(x, ln1_w, ln1_b, attn_w, attn_b, proj_w, proj_b,
           ln2_w, ln2_b, fc_w, fc_b, cproj_w, cproj_b):
    global _CACHED_NC
    x = np.asarray(x)
    B = x.shape[0]
    assert B == N_CORES and x.shape[1] == T and x.shape[2] == C
    if _CACHED_NC is None:
        _CACHED_NC = build()
    nc = _CACHED_NC
    args = [np.asarray(a, dtype=np.float32)
            for a in (attn_w, attn_b, proj_w, proj_b, fc_w, fc_b,
                      cproj_w, cproj_b, ln1_w, ln1_b, ln2_w, ln2_b)]
    (attn_w, attn_b, proj_w, proj_b, fc_w, fc_b,
     cproj_w, cproj_b, ln1_w, ln1_b, ln2_w, ln2_b) = args
    in_maps = [
        host_inputs(x[b], attn_w, attn_b, proj_w, proj_b, fc_w, fc_b,
                    cproj_w, cproj_b, ln1_w, ln1_b, ln2_w, ln2_b)
        for b in range(B)
    ]
    res = bass_utils.run_bass_kernel_spmd(
        nc, in_maps, core_ids=list(range(N_CORES))
    )
    out = np.empty((B, T, C), np.float32)
    for b in range(B):
        out[b] = unpack_output(res.results[b])
    return out
